# revision 8
# baseline (speedup 1.0000x reference)
"""Trainium2 Bass kernel for nn_DecoderLayer (LSTM cell + Bahdanau attention + SwiGLU FFN).

Strategy: pure data-parallel over batch B=1024 across 8 cores (128 rows each, no
collectives). Host pre-transposes weights / packs keys so every matmul operand
streams from DRAM in its natural layout (contraction dim on partitions). All
matmul operands are bf16 (fp32 PSUM accumulation); LayerNorm / softmax / LSTM
elementwise math is fp32.
"""

import sys

for _p in ("/opt/trn_rl_repo",):
    if _p not in sys.path:
        sys.path.insert(0, _p)

from contextlib import ExitStack

import ml_dtypes
import numpy as np

import concourse.bass as bass
import concourse.tile as tile
from concourse import bacc, masks, mybir
from concourse.bass_utils import run_bass_kernel_spmd

dt = mybir.dt
ts = bass.ts
AF = mybir.ActivationFunctionType
ALU = mybir.AluOpType

B, S, H, F = 1024, 128, 1024, 4096
NCORES = 8
BC = B // NCORES          # 128 batch rows per core
P = 128                   # partitions
KH = H // P               # 8 contraction chunks over H
GB = 4                    # batch rows per attention group
NG = BC // GB             # 32 groups
H4 = 4 * H
EPS = 1e-5

BF16 = dt.bfloat16
F32 = dt.float32
F32R = dt.float32r

TRACE = False
LAST_RESULT = None


def _f32r(ap):
    return ap.bitcast(F32R)


def build_program():
    nc = bacc.Bacc("TRN2", target_bir_lowering=False, debug=False,
                   enable_asserts=True, num_devices=NCORES)

    def din(name, shape, dtype):
        return nc.dram_tensor(name, list(shape), dtype, kind="ExternalInput").ap()

    def dout(name, shape, dtype):
        return nc.dram_tensor(name, list(shape), dtype, kind="ExternalOutput").ap()

    x_d = din("x", (BC, H), F32)
    h0T_d = din("h0T", (H, BC), BF16)
    c0_d = din("c0", (BC, H), F32)
    keysT_d = din("keysT", (NG, H, GB, S), BF16)
    keysN_d = din("keysN", (BC, S, H), BF16)
    WihT_d = din("WihT", (H, H4), BF16)
    WhhT_d = din("WhhT", (H, H4), BF16)
    biasg_d = din("biasg", (1, H4), BF16)
    WaT_d = din("WaT", (H, H), BF16)
    Wab_d = din("Wab", (1, H), BF16)
    UaT_d = din("UaT", (H, H), BF16)
    Va8_d = din("Va8", (P, KH), BF16)
    w1T_d = din("w1T", (H, F), BF16)
    w3T_d = din("w3T", (H, F), BF16)
    w2T_d = din("w2T", (F, H), BF16)
    lng_d = [din(f"ln{i}g", (P, H), F32) for i in (1, 2, 3)]
    lnb_d = [din(f"ln{i}b", (P, H), F32) for i in (1, 2, 3)]

    out_d = dout("out", (BC, H), F32)
    newh_d = dout("new_h", (BC, H), F32)
    cnew_d = dout("c_new", (BC, H), F32)

    sc_scratch_d = nc.dram_tensor("sc_scratch", [BC * S], F32).ap()

    with tile.TileContext(nc) as tc, ExitStack() as ctx:
        cpool = ctx.enter_context(tc.tile_pool(name="const", bufs=1))
        gpool = ctx.enter_context(tc.tile_pool(name="glob", bufs=1))
        spool = ctx.enter_context(tc.tile_pool(name="small", bufs=1))
        wres = ctx.enter_context(tc.tile_pool(name="wres", bufs=1))
        wstream = ctx.enter_context(tc.tile_pool(name="wstream", bufs=12))
        ktpool = ctx.enter_context(tc.tile_pool(name="ktpool", bufs=16))
        epool = ctx.enter_context(tc.tile_pool(name="epool", bufs=10))
        knpool = ctx.enter_context(tc.tile_pool(name="knpool", bufs=4))
        vtpool = ctx.enter_context(tc.tile_pool(name="vtpool", bufs=3))
        opool = ctx.enter_context(tc.tile_pool(name="opool", bufs=2))

        # ---- constants ----
        ident_bf = cpool.tile([P, P], BF16, tag="ident_bf")
        masks.make_identity(nc, ident_bf[:])
        ident_f = cpool.tile([P, P], F32, tag="ident_f")
        masks.make_identity(nc, ident_f[:])
        ones_f = cpool.tile([1, P], BF16, tag="ones_f")
        nc.vector.memset(ones_f[:], 1.0)
        zcol = cpool.tile([P, 1], F32, tag="zcol")
        nc.vector.memset(zcol[:], 0.0)
        ecol = cpool.tile([P, 1], F32, tag="ecol")
        nc.vector.memset(ecol[:], EPS)
        biasg_t = cpool.tile([1, H4], BF16, tag="biasg")
        nc.sync.dma_start(out=biasg_t[:], in_=biasg_d[:])
        wab_t = cpool.tile([1, H], BF16, tag="wab")
        nc.sync.dma_start(out=wab_t[:], in_=Wab_d[:])
        va_t = cpool.tile([P, KH], BF16, tag="va")
        nc.sync.dma_start(out=va_t[:], in_=Va8_d[:])

        lng_t = cpool.tile([P, H], F32, tag="lng")
        lnb_t = cpool.tile([P, H], F32, tag="lnb")

        # resident weight for the k projection: [p, kchunk, col]
        UaT_sb = wres.tile([P, KH, H], BF16, tag="UaT")
        nc.sync.dma_start(out=UaT_sb[:], in_=UaT_d[:].rearrange("(k p) m -> p k m", p=P))

        # ---- global activations ----
        x_t = gpool.tile([BC, H], F32, tag="x")
        nc.sync.dma_start(out=x_t[:], in_=x_d[:])
        c0_t = gpool.tile([BC, H], F32, tag="c0")
        nc.sync.dma_start(out=c0_t[:], in_=c0_d[:])
        h0T_t = gpool.tile([P, KH, BC], BF16, tag="h0T")
        nc.sync.dma_start(out=h0T_t[:], in_=h0T_d[:].rearrange("(k p) b -> p k b", p=P))

        def layer_norm(xin, ln_idx, out_f32, out_bf):
            """LN over free dim; writes fp32 result and optional bf16 copy."""
            nc.sync.dma_start(out=lng_t[:], in_=lng_d[ln_idx][:])
            nc.sync.dma_start(out=lnb_t[:], in_=lnb_d[ln_idx][:])
            tg = f"ln{ln_idx}"
            s1 = spool.tile([P, 1], F32, tag=tg + "s1")
            nc.vector.tensor_reduce(out=s1[:], in_=xin, axis=mybir.AxisListType.X,
                                    op=ALU.add)
            nm = spool.tile([P, 1], F32, tag=tg + "nm")
            nc.vector.tensor_scalar_mul(nm[:], s1[:], -1.0 / H)
            xc = gpool.tile([BC, H], F32, tag="ln_xc")
            nc.vector.tensor_scalar(xc[:], xin, nm[:], None, ALU.add)
            sq = gpool.tile([BC, H], F32, tag="scratch2", name="sq")
            ss = spool.tile([P, 1], F32, tag=tg + "ss")
            nc.scalar.activation(sq[:], xc[:], AF.Square, bias=zcol[:], accum_out=ss[:])
            sd = spool.tile([P, 1], F32, tag=tg + "sd")
            nc.scalar.activation(sd[:], ss[:], AF.Sqrt, bias=ecol[:], scale=1.0 / H)
            rstd = spool.tile([P, 1], F32, tag=tg + "rstd")
            nc.vector.reciprocal(rstd[:], sd[:])
            nc.vector.tensor_scalar(xc[:], xc[:], rstd[:], None, ALU.mult)
            nc.vector.tensor_tensor(out=xc[:], in0=xc[:], in1=lng_t[:], op=ALU.mult)
            if out_f32 is not None:
                nc.vector.tensor_tensor(out=out_f32, in0=xc[:], in1=lnb_t[:], op=ALU.add)
                if out_bf is not None:
                    nc.vector.tensor_copy(out_bf, out_f32)
            else:
                nc.vector.tensor_tensor(out=out_bf, in0=xc[:], in1=lnb_t[:], op=ALU.add)

        def pe_transpose_to(dst, src128, ident, tp_pool, dtype):
            t = tp_pool.tile([P, P], dtype, tag="tp")
            nc.tensor.matmul(t[:], src128, ident[:], is_transpose=True)
            nc.vector.tensor_copy(dst, t[:])

        # ================= Phase 1: LN1 + transpose =================
        xnorm_bf = gpool.tile([BC, H], BF16, tag="xnorm_bf")
        layer_norm(x_t[:], 0, None, xnorm_bf[:])

        xnT = gpool.tile([P, KH, BC], BF16, tag="xnT")
        with tc.tile_pool(name="tp1", bufs=2, space="PSUM") as tp1:
            for k in range(KH):
                pe_transpose_to(xnT[:, k, :], xnorm_bf[:, ts(k, P)], ident_bf, tp1, BF16)

            # ============= Phase 2: LSTM gates =============
            si = gpool.tile([BC, H], F32, tag="si")
            sf = gpool.tile([BC, H], F32, tag="sf")
            tg_ = gpool.tile([BC, H], F32, tag="tg")
            so = gpool.tile([BC, H], F32, tag="so")
            gate_sb = [si, sf, tg_, so]
            with tc.tile_pool(name="gpsum", bufs=4, space="PSUM") as gpsum:
                for half in range(2):
                    pss = [gpsum.tile([P, 512], F32, tag="g", name=f"gps{half}_{i}") for i in range(4)]
                    for k in range(KH):
                        for n in range(4):
                            nn = half * 4 + n
                            wt = wstream.tile([P, 512], BF16, tag="w")
                            nc.sync.dma_start(out=wt[:], in_=WihT_d[ts(k, P), ts(nn, 512)])
                            nc.tensor.matmul(pss[n][:], xnT[:, k, :], wt[:],
                                             start=(k == 0), stop=False)
                    for k in range(KH):
                        for n in range(4):
                            nn = half * 4 + n
                            wt = wstream.tile([P, 512], BF16, tag="w")
                            nc.sync.dma_start(out=wt[:], in_=WhhT_d[ts(k, P), ts(nn, 512)])
                            nc.tensor.matmul(pss[n][:], h0T_t[:, k, :], wt[:],
                                             start=False, stop=False)
                    for n in range(4):
                        nn = half * 4 + n
                        nc.tensor.matmul(pss[n][:], ones_f[:],
                                         biasg_t[:, ts(nn, 512)],
                                         start=False, stop=True)
                        gate = nn // 2   # 0:i 1:f 2:g 3:o
                        func = AF.Tanh if gate == 2 else AF.Sigmoid
                        nc.scalar.activation(gate_sb[gate][:, ts(nn % 2, 512)],
                                             pss[n][:], func, bias=zcol[:])

            # LSTM cell elementwise
            cn_t = gpool.tile([BC, H], F32, tag="cn")
            nc.vector.tensor_tensor(out=cn_t[:], in0=sf[:], in1=c0_t[:], op=ALU.mult)
            t2 = gpool.tile([BC, H], F32, tag="scratch2", name="t2")
            nc.vector.tensor_tensor(out=t2[:], in0=si[:], in1=tg_[:], op=ALU.mult)
            nc.vector.tensor_tensor(out=cn_t[:], in0=cn_t[:], in1=t2[:], op=ALU.add)
            nc.sync.dma_start(out=cnew_d[:], in_=cn_t[:])
            tcn = gpool.tile([BC, H], F32, tag="scratch2", name="tcn")
            nc.scalar.activation(tcn[:], cn_t[:], AF.Tanh, bias=zcol[:])
            newh_t = gpool.tile([BC, H], F32, tag="newh")
            nc.vector.tensor_tensor(out=newh_t[:], in0=so[:], in1=tcn[:], op=ALU.mult)
            nc.vector.tensor_tensor(out=newh_t[:], in0=x_t[:], in1=newh_t[:], op=ALU.add)
            nc.sync.dma_start(out=newh_d[:], in_=newh_t[:])

            # ============= Phase 3: LN2 + transpose =============
            hnorm_f = gpool.tile([BC, H], F32, tag="hnorm_f")
            hnorm_bf = gpool.tile([BC, H], BF16, tag="hnorm_bf")
            layer_norm(newh_t[:], 1, hnorm_f[:], hnorm_bf[:])
            hnT = gpool.tile([P, KH, BC], BF16, tag="hnT")
            for k in range(KH):
                pe_transpose_to(hnT[:, k, :], hnorm_bf[:, ts(k, P)], ident_bf, tp1, BF16)

        # ============= Phase 4: qT = Wa @ h_norm^T + Wa_b (transposed layout) ====
        qT = gpool.tile([P, KH, BC], F32, tag="qT")
        with tc.tile_pool(name="qpsum", bufs=2, space="PSUM") as qpsum:
            for m in range(KH):
                ps = qpsum.tile([P, BC], F32, tag="q")
                for k in range(KH):
                    wt = wstream.tile([P, P], BF16, tag="w", name="wa_t")
                    nc.sync.dma_start(out=wt[:], in_=WaT_d[ts(k, P), ts(m, P)])
                    nc.tensor.matmul(ps[:], wt[:], hnT[:, k, :],
                                     start=(k == 0), stop=False)
                nc.tensor.matmul(ps[:], wab_t[:, ts(m, P)], ones_f[:],
                                 start=False, stop=True)
                nc.vector.tensor_copy(qT[:, m, :], ps[:])

        # ============= Phase 5: attention scores =============
        with tc.tile_pool(name="ktpsum", bufs=5, space="PSUM") as ktpsum, \
             tc.tile_pool(name="scpsum", bufs=2, space="PSUM") as scpsum:
            for g in range(NG):
                kts = []
                for k in range(KH):
                    kt = ktpool.tile([P, GB, S], BF16, tag="kt")
                    nc.sync.dma_start(out=kt[:], in_=keysT_d[g, ts(k, P), :, :])
                    kts.append(kt)
                sc_ps = scpsum.tile([1, GB * S], F32, tag="sc")
                for m in range(KH):
                    ps = ktpsum.tile([P, GB * S], F32, tag="ktp")
                    for k in range(KH):
                        nc.tensor.matmul(ps[:], UaT_sb[:, k, ts(m, P)], kts[k][:],
                                         start=(k == 0), stop=(k == KH - 1))
                    e_m = epool.tile([P, GB, S], BF16, tag="e")
                    for j in range(GB):
                        b = g * GB + j
                        nc.scalar.activation(e_m[:, j, :], ps[:, ts(j, S)], AF.Tanh,
                                             bias=qT[:, m, b:b + 1])
                    nc.tensor.matmul(sc_ps[:], va_t[:, m:m + 1], e_m[:],
                                     start=(m == 0), stop=(m == KH - 1))
                scb = opool.tile([1, GB * S], F32, tag="scb", name="scb")
                nc.vector.tensor_copy(scb[:], sc_ps[:])
                nc.sync.dma_start(out=sc_scratch_d[ts(g, GB * S)], in_=scb[:])

        # ============= Phase 6: softmax =============
        sc2 = gpool.tile([BC, S], F32, tag="sc2")
        nc.sync.dma_start(out=sc2[:], in_=sc_scratch_d[:].rearrange("(b s) -> b s", b=BC))
        mx = spool.tile([P, 1], F32, tag="mx")
        nc.vector.tensor_reduce(out=mx[:], in_=sc2[:], axis=mybir.AxisListType.X,
                                op=ALU.max)
        nmx = spool.tile([P, 1], F32, tag="nmx")
        nc.vector.tensor_scalar_mul(nmx[:], mx[:], -1.0)
        wsm = gpool.tile([BC, S], F32, tag="wsm")
        sume = spool.tile([P, 1], F32, tag="sume")
        nc.scalar.activation(wsm[:], sc2[:], AF.Exp, bias=nmx[:], accum_out=sume[:])
        rse = spool.tile([P, 1], F32, tag="rse")
        nc.vector.reciprocal(rse[:], sume[:])
        wsm_bf = gpool.tile([BC, S], BF16, tag="wsm_bf")
        nc.vector.tensor_scalar(wsm_bf[:], wsm[:], rse[:], None, ALU.mult)
        wT_sb = gpool.tile([S, BC], BF16, tag="wT")
        with tc.tile_pool(name="tp2", bufs=2, space="PSUM") as tp2:
            pe_transpose_to(wT_sb[:], wsm_bf[:], ident_bf, tp2, BF16)

        # ============= Phase 7: context =============
        ctxT = gpool.tile([P, KH, BC], F32, tag="ctxT")
        with tc.tile_pool(name="cpsum", bufs=1, space="PSUM") as cpsum:
            cts = [cpsum.tile([P, BC], F32, tag=f"ct{m}", name=f"ct{m}") for m in range(KH)]
            for b in range(BC):
                kn = knpool.tile([S, H], BF16, tag="kn")
                nc.sync.dma_start(out=kn[:], in_=keysN_d[b, :, :])
                for m in range(KH):
                    nc.tensor.matmul(cts[m][:, b:b + 1], kn[:, ts(m, P)],
                                     wT_sb[:, b:b + 1], start=True, stop=True)
            for m in range(KH):
                nc.vector.tensor_copy(ctxT[:, m, :], cts[m][:])

        ctxn = gpool.tile([BC, H], F32, tag="ctxn")
        with tc.tile_pool(name="tp3", bufs=2, space="PSUM") as tp3:
            for m in range(KH):
                pe_transpose_to(ctxn[:, ts(m, P)], ctxT[:, m, :], ident_f, tp3, F32)

            # ============= Phase 8: LN3 =============
            nc.vector.tensor_tensor(out=ctxn[:], in0=hnorm_f[:], in1=ctxn[:],
                                    op=ALU.add)
            attnh_bf = gpool.tile([BC, H], BF16, tag="attnh_bf")
            layer_norm(ctxn[:], 2, None, attnh_bf[:])
            ahT = gpool.tile([P, KH, BC], BF16, tag="ahT")
            for k in range(KH):
                pe_transpose_to(ahT[:, k, :], attnh_bf[:, ts(k, P)], ident_bf, tp3, BF16)

        # ============= Phase 9: SwiGLU FFN =============
        NF = F // 512  # 8 chunks of the ffn dim
        with tc.tile_pool(name="fpsum", bufs=1, space="PSUM") as fpsum, \
             tc.tile_pool(name="upsum", bufs=2, space="PSUM") as upsum, \
             tc.tile_pool(name="tpsum", bufs=2, space="PSUM") as tpsum:
            ffd = [fpsum.tile([P, 512], F32, tag=f"ffd{h2}", name=f"ffd{h2}") for h2 in range(2)]
            for n in range(NF):
                u1 = upsum.tile([P, 512], F32, tag="u1")
                u3 = upsum.tile([P, 512], F32, tag="u3")
                for k in range(KH):
                    wt = wstream.tile([P, 512], BF16, tag="w")
                    nc.sync.dma_start(out=wt[:], in_=w1T_d[ts(k, P), ts(n, 512)])
                    nc.tensor.matmul(u1[:], ahT[:, k, :], wt[:],
                                     start=(k == 0), stop=(k == KH - 1))
                for k in range(KH):
                    wt = wstream.tile([P, 512], BF16, tag="w")
                    nc.sync.dma_start(out=wt[:], in_=w3T_d[ts(k, P), ts(n, 512)])
                    nc.tensor.matmul(u3[:], ahT[:, k, :], wt[:],
                                     start=(k == 0), stop=(k == KH - 1))
                sg = opool.tile([P, 512], F32, tag="sg")
                nc.scalar.activation(sg[:], u1[:], AF.Sigmoid, bias=zcol[:])
                v1 = opool.tile([P, 512], F32, tag="v1")
                nc.vector.tensor_tensor(out=v1[:], in0=u1[:], in1=sg[:], op=ALU.mult)
                v_bf = opool.tile([P, 512], BF16, tag="v_bf")
                nc.vector.tensor_tensor(out=v_bf[:], in0=v1[:], in1=u3[:], op=ALU.mult)
                for c in range(4):
                    f_idx = n * 4 + c
                    tp = tpsum.tile([P, P], BF16, tag="vtp")
                    nc.tensor.matmul(tp[:], v_bf[:, ts(c, P)], ident_bf[:],
                                     is_transpose=True)
                    vT = vtpool.tile([P, P], BF16, tag="vT")
                    nc.vector.tensor_copy(vT[:], tp[:])
                    for h2 in range(2):
                        wt = wstream.tile([P, 512], BF16, tag="w")
                        nc.sync.dma_start(out=wt[:], in_=w2T_d[ts(f_idx, P), ts(h2, 512)])
                        nc.tensor.matmul(ffd[h2][:], vT[:], wt[:],
                                         start=(f_idx == 0), stop=(f_idx == F // P - 1),
                                         skip_group_check=True)
            for h2 in range(2):
                ot = opool.tile([P, 512], F32, tag="ot")
                nc.vector.tensor_tensor(out=ot[:], in0=ffd[h2][:],
                                        in1=newh_t[:, ts(h2, 512)], op=ALU.add)
                nc.sync.dma_start(out=out_d[:, ts(h2, 512)], in_=ot[:])

    nc.compile()
    return nc


def prepare_inputs(inputs):
    """Host-side slicing / transposition / dtype casts. Returns per-core in_maps."""
    bf = ml_dtypes.bfloat16
    f32 = np.float32

    def c(a, dtype):
        return np.ascontiguousarray(a, dtype=dtype)

    x = np.asarray(inputs["x"], f32)
    h0 = np.asarray(inputs["h0"], f32)
    c0 = np.asarray(inputs["c0"], f32)
    keys = np.asarray(inputs["keys"], f32)

    shared = {
        "WihT": c(np.asarray(inputs["W_ih"]).T, bf),
        "WhhT": c(np.asarray(inputs["W_hh"]).T, bf),
        "biasg": c((np.asarray(inputs["b_ih"]) + np.asarray(inputs["b_hh"]))[None, :], bf),
        "WaT": c(np.asarray(inputs["Wa"]).T, bf),
        "Wab": c(np.asarray(inputs["Wa_b"])[None, :], bf),
        "UaT": c(np.asarray(inputs["Ua"]).T, bf),
        "Va8": c(np.asarray(inputs["Va"])[0].reshape(KH, P).T, bf),
        "w1T": c(np.asarray(inputs["w1"]).T, bf),
        "w3T": c(np.asarray(inputs["w3"]).T, bf),
        "w2T": c(np.asarray(inputs["w2"]).T, bf),
    }
    for i in (1, 2, 3):
        shared[f"ln{i}g"] = c(np.broadcast_to(np.asarray(inputs[f"ln{i}_g"]), (P, H)), f32)
        shared[f"ln{i}b"] = c(np.broadcast_to(np.asarray(inputs[f"ln{i}_b"]), (P, H)), f32)

    in_maps = []
    for core in range(NCORES):
        sl = slice(core * BC, (core + 1) * BC)
        kc = keys[sl]                                    # [BC, S, H]
        keysT = kc.reshape(NG, GB, S, H).transpose(0, 3, 1, 2)   # [NG, H, GB, S]
        m = dict(shared)
        m["x"] = c(x[sl], f32)
        m["h0T"] = c(h0[sl].T, bf)
        m["c0"] = c(c0[sl], f32)
        m["keysT"] = c(keysT, bf)
        m["keysN"] = c(kc, bf)
        in_maps.append(m)
    return in_maps


_PROGRAM = None


def kernel(**inputs):
    global _PROGRAM, LAST_RESULT
    if _PROGRAM is None:
        _PROGRAM = build_program()
    in_maps = prepare_inputs(inputs)
    res = run_bass_kernel_spmd(_PROGRAM, in_maps, list(range(NCORES)), trace=TRACE)
    LAST_RESULT = res
    outs = np.concatenate([np.asarray(r["out"]) for r in res.results], axis=0)
    newh = np.concatenate([np.asarray(r["new_h"]) for r in res.results], axis=0)
    cnew = np.concatenate([np.asarray(r["c_new"]) for r in res.results], axis=0)
    return outs.astype(np.float32), (newh.astype(np.float32), cnew.astype(np.float32))


# revision 9
# speedup vs baseline: 1.0205x; 1.0205x over previous
"""Trainium2 Bass kernel for nn_DecoderLayer (LSTM cell + Bahdanau attention + SwiGLU FFN).

Strategy: pure data-parallel over batch B=1024 across 8 cores (128 rows each, no
collectives). Host pre-transposes weights / packs keys so every matmul operand
streams from DRAM in its natural layout (contraction dim on partitions). All
matmul operands are bf16 (fp32 PSUM accumulation); LayerNorm / softmax / LSTM
elementwise math is fp32.
"""

import sys

for _p in ("/opt/trn_rl_repo",):
    if _p not in sys.path:
        sys.path.insert(0, _p)

from contextlib import ExitStack

import ml_dtypes
import numpy as np

import concourse.bass as bass
import concourse.tile as tile
from concourse import bacc, masks, mybir
from concourse.bass_utils import run_bass_kernel_spmd

dt = mybir.dt
ts = bass.ts
AF = mybir.ActivationFunctionType
ALU = mybir.AluOpType

B, S, H, F = 1024, 128, 1024, 4096
NCORES = 8
BC = B // NCORES          # 128 batch rows per core
P = 128                   # partitions
KH = H // P               # 8 contraction chunks over H
GB = 4                    # batch rows per attention group
NG = BC // GB             # 32 groups
H4 = 4 * H
EPS = 1e-5

BF16 = dt.bfloat16
F32 = dt.float32
F32R = dt.float32r

TRACE = False
LAST_RESULT = None


def _f32r(ap):
    return ap.bitcast(F32R)


def build_program():
    nc = bacc.Bacc("TRN2", target_bir_lowering=False, debug=False,
                   enable_asserts=True, num_devices=NCORES)

    def din(name, shape, dtype):
        return nc.dram_tensor(name, list(shape), dtype, kind="ExternalInput").ap()

    def dout(name, shape, dtype):
        return nc.dram_tensor(name, list(shape), dtype, kind="ExternalOutput").ap()

    x_d = din("x", (BC, H), F32)
    h0T_d = din("h0T", (H, BC), BF16)
    c0_d = din("c0", (BC, H), F32)
    keysT_d = din("keysT", (NG, H, GB, S), BF16)
    keysN_d = din("keysN", (BC, S, H), BF16)
    WihT_d = din("WihT", (H, H4), BF16)
    WhhT_d = din("WhhT", (H, H4), BF16)
    biasg_d = din("biasg", (1, H4), BF16)
    WaT_d = din("WaT", (H, H), BF16)
    Wab_d = din("Wab", (1, H), BF16)
    UaT_d = din("UaT", (H, H), BF16)
    Va8_d = din("Va8", (P, KH), BF16)
    w1T_d = din("w1T", (H, F), BF16)
    w3T_d = din("w3T", (H, F), BF16)
    w2T_d = din("w2T", (F, H), BF16)
    lng_d = [din(f"ln{i}g", (P, H), F32) for i in (1, 2, 3)]
    lnb_d = [din(f"ln{i}b", (P, H), F32) for i in (1, 2, 3)]

    out_d = dout("out", (BC, H), F32)
    newh_d = dout("new_h", (BC, H), F32)
    cnew_d = dout("c_new", (BC, H), F32)

    sc_scratch_d = nc.dram_tensor("sc_scratch", [BC * S], F32).ap()

    with tile.TileContext(nc) as tc, ExitStack() as ctx:
        cpool = ctx.enter_context(tc.tile_pool(name="const", bufs=1))
        gpool = ctx.enter_context(tc.tile_pool(name="glob", bufs=1))
        spool = ctx.enter_context(tc.tile_pool(name="small", bufs=1))
        wres = ctx.enter_context(tc.tile_pool(name="wres", bufs=1))
        wstream = ctx.enter_context(tc.tile_pool(name="wstream", bufs=16))
        ktpool = ctx.enter_context(tc.tile_pool(name="ktpool", bufs=12))
        epool = ctx.enter_context(tc.tile_pool(name="epool", bufs=8))
        knpool = ctx.enter_context(tc.tile_pool(name="knpool", bufs=3))
        vtpool = ctx.enter_context(tc.tile_pool(name="vtpool", bufs=3))
        opool = ctx.enter_context(tc.tile_pool(name="opool", bufs=2))

        # ---- constants ----
        ident_bf = cpool.tile([P, P], BF16, tag="ident_bf")
        masks.make_identity(nc, ident_bf[:])
        ident_f = cpool.tile([P, P], F32, tag="ident_f")
        masks.make_identity(nc, ident_f[:])
        ones_f = cpool.tile([1, P], BF16, tag="ones_f")
        nc.vector.memset(ones_f[:], 1.0)
        zcol = cpool.tile([P, 1], F32, tag="zcol")
        nc.vector.memset(zcol[:], 0.0)
        ecol = cpool.tile([P, 1], F32, tag="ecol")
        nc.vector.memset(ecol[:], EPS)
        biasg_t = cpool.tile([1, H4], BF16, tag="biasg")
        nc.sync.dma_start(out=biasg_t[:], in_=biasg_d[:])
        wab_t = cpool.tile([1, H], BF16, tag="wab")
        nc.sync.dma_start(out=wab_t[:], in_=Wab_d[:])
        va_t = cpool.tile([P, KH], BF16, tag="va")
        nc.sync.dma_start(out=va_t[:], in_=Va8_d[:])

        lng_t = cpool.tile([P, H], F32, tag="lng")
        lnb_t = cpool.tile([P, H], F32, tag="lnb")

        # resident weight for the k projection: [p, kchunk, col]
        UaT_sb = wres.tile([P, KH, H], BF16, tag="UaT")
        nc.sync.dma_start(out=UaT_sb[:], in_=UaT_d[:].rearrange("(k p) m -> p k m", p=P))

        # ---- global activations ----
        x_t = gpool.tile([BC, H], F32, tag="x")
        nc.sync.dma_start(out=x_t[:], in_=x_d[:])
        c0_t = gpool.tile([BC, H], F32, tag="c0")
        nc.sync.dma_start(out=c0_t[:], in_=c0_d[:])
        h0T_t = gpool.tile([P, KH, BC], BF16, tag="h0T")
        nc.sync.dma_start(out=h0T_t[:], in_=h0T_d[:].rearrange("(k p) b -> p k b", p=P))

        def layer_norm(xin, ln_idx, out_f32, out_bf):
            """LN over free dim; writes fp32 result and optional bf16 copy."""
            nc.sync.dma_start(out=lng_t[:], in_=lng_d[ln_idx][:])
            nc.sync.dma_start(out=lnb_t[:], in_=lnb_d[ln_idx][:])
            tg = f"ln{ln_idx}"
            s1 = spool.tile([P, 1], F32, tag=tg + "s1")
            nc.vector.tensor_reduce(out=s1[:], in_=xin, axis=mybir.AxisListType.X,
                                    op=ALU.add)
            nm = spool.tile([P, 1], F32, tag=tg + "nm")
            nc.vector.tensor_scalar_mul(nm[:], s1[:], -1.0 / H)
            xc = gpool.tile([BC, H], F32, tag="ln_xc")
            nc.vector.tensor_scalar(xc[:], xin, nm[:], None, ALU.add)
            sq = gpool.tile([BC, H], F32, tag="scratch2", name="sq")
            ss = spool.tile([P, 1], F32, tag=tg + "ss")
            nc.scalar.activation(sq[:], xc[:], AF.Square, bias=zcol[:], accum_out=ss[:])
            sd = spool.tile([P, 1], F32, tag=tg + "sd")
            nc.scalar.activation(sd[:], ss[:], AF.Sqrt, bias=ecol[:], scale=1.0 / H)
            rstd = spool.tile([P, 1], F32, tag=tg + "rstd")
            nc.vector.reciprocal(rstd[:], sd[:])
            nc.vector.tensor_scalar(xc[:], xc[:], rstd[:], None, ALU.mult)
            nc.vector.tensor_tensor(out=xc[:], in0=xc[:], in1=lng_t[:], op=ALU.mult)
            if out_f32 is not None:
                nc.vector.tensor_tensor(out=out_f32, in0=xc[:], in1=lnb_t[:], op=ALU.add)
                if out_bf is not None:
                    nc.vector.tensor_copy(out_bf, out_f32)
            else:
                nc.vector.tensor_tensor(out=out_bf, in0=xc[:], in1=lnb_t[:], op=ALU.add)

        def pe_transpose_to(dst, src128, ident, tp_pool, dtype):
            t = tp_pool.tile([P, P], dtype, tag="tp")
            nc.tensor.matmul(t[:], src128, ident[:], is_transpose=True)
            nc.vector.tensor_copy(dst, t[:])

        # ================= Phase 1: LN1 + transpose =================
        xnorm_bf = gpool.tile([BC, H], BF16, tag="xnorm_bf")
        layer_norm(x_t[:], 0, None, xnorm_bf[:])

        xnT = gpool.tile([P, KH, BC], BF16, tag="xnT")
        with tc.tile_pool(name="tp1", bufs=2, space="PSUM") as tp1:
            for k in range(KH):
                pe_transpose_to(xnT[:, k, :], xnorm_bf[:, ts(k, P)], ident_bf, tp1, BF16)

            # ============= Phase 2: LSTM gates =============
            si = gpool.tile([BC, H], F32, tag="si")
            sf = gpool.tile([BC, H], F32, tag="sf")
            tg_ = gpool.tile([BC, H], F32, tag="tg")
            so = gpool.tile([BC, H], F32, tag="so")
            gate_sb = [si, sf, tg_, so]
            with tc.tile_pool(name="gpsum", bufs=4, space="PSUM") as gpsum:
                for half in range(2):
                    pss = [gpsum.tile([P, 512], F32, tag="g", name=f"gps{half}_{i}") for i in range(4)]
                    for k in range(KH):
                        for n in range(4):
                            nn = half * 4 + n
                            wt = wstream.tile([P, 512], BF16, tag="w")
                            nc.sync.dma_start(out=wt[:], in_=WihT_d[ts(k, P), ts(nn, 512)])
                            nc.tensor.matmul(pss[n][:], xnT[:, k, :], wt[:],
                                             start=(k == 0), stop=False)
                    for k in range(KH):
                        for n in range(4):
                            nn = half * 4 + n
                            wt = wstream.tile([P, 512], BF16, tag="w")
                            nc.sync.dma_start(out=wt[:], in_=WhhT_d[ts(k, P), ts(nn, 512)])
                            nc.tensor.matmul(pss[n][:], h0T_t[:, k, :], wt[:],
                                             start=False, stop=False)
                    for n in range(4):
                        nn = half * 4 + n
                        nc.tensor.matmul(pss[n][:], ones_f[:],
                                         biasg_t[:, ts(nn, 512)],
                                         start=False, stop=True)
                        gate = nn // 2   # 0:i 1:f 2:g 3:o
                        func = AF.Tanh if gate == 2 else AF.Sigmoid
                        nc.scalar.activation(gate_sb[gate][:, ts(nn % 2, 512)],
                                             pss[n][:], func, bias=zcol[:])

            # LSTM cell elementwise
            cn_t = gpool.tile([BC, H], F32, tag="cn")
            nc.vector.tensor_tensor(out=cn_t[:], in0=sf[:], in1=c0_t[:], op=ALU.mult)
            t2 = gpool.tile([BC, H], F32, tag="scratch2", name="t2")
            nc.vector.tensor_tensor(out=t2[:], in0=si[:], in1=tg_[:], op=ALU.mult)
            nc.vector.tensor_tensor(out=cn_t[:], in0=cn_t[:], in1=t2[:], op=ALU.add)
            nc.sync.dma_start(out=cnew_d[:], in_=cn_t[:])
            tcn = gpool.tile([BC, H], F32, tag="scratch2", name="tcn")
            nc.scalar.activation(tcn[:], cn_t[:], AF.Tanh, bias=zcol[:])
            newh_t = gpool.tile([BC, H], F32, tag="newh")
            nc.vector.tensor_tensor(out=newh_t[:], in0=so[:], in1=tcn[:], op=ALU.mult)
            nc.vector.tensor_tensor(out=newh_t[:], in0=x_t[:], in1=newh_t[:], op=ALU.add)
            nc.sync.dma_start(out=newh_d[:], in_=newh_t[:])

            # ============= Phase 3: LN2 + transpose =============
            hnorm_f = gpool.tile([BC, H], F32, tag="hnorm_f")
            hnorm_bf = gpool.tile([BC, H], BF16, tag="hnorm_bf")
            layer_norm(newh_t[:], 1, hnorm_f[:], hnorm_bf[:])
            hnT = gpool.tile([P, KH, BC], BF16, tag="hnT")
            for k in range(KH):
                pe_transpose_to(hnT[:, k, :], hnorm_bf[:, ts(k, P)], ident_bf, tp1, BF16)

        # ============= Phase 4: qT = Wa @ h_norm^T + Wa_b (transposed layout) ====
        qT = gpool.tile([P, KH, BC], F32, tag="si", name="qT")
        with tc.tile_pool(name="qpsum", bufs=2, space="PSUM") as qpsum:
            for m in range(KH):
                ps = qpsum.tile([P, BC], F32, tag="q")
                for k in range(KH):
                    wt = wstream.tile([P, P], BF16, tag="w", name="wa_t")
                    nc.sync.dma_start(out=wt[:], in_=WaT_d[ts(k, P), ts(m, P)])
                    nc.tensor.matmul(ps[:], wt[:], hnT[:, k, :],
                                     start=(k == 0), stop=False)
                nc.tensor.matmul(ps[:], wab_t[:, ts(m, P)], ones_f[:],
                                 start=False, stop=True)
                nc.vector.tensor_copy(qT[:, m, :], ps[:])

        # ============= Phase 5: attention scores =============
        with tc.tile_pool(name="ktpsum", bufs=5, space="PSUM") as ktpsum, \
             tc.tile_pool(name="scpsum", bufs=2, space="PSUM") as scpsum:
            for g in range(NG):
                kts = []
                for k in range(KH):
                    kt = ktpool.tile([P, GB, S], BF16, tag="kt")
                    nc.sync.dma_start(out=kt[:], in_=keysT_d[g, ts(k, P), :, :])
                    kts.append(kt)
                sc_ps = scpsum.tile([1, GB * S], F32, tag="sc")
                for m in range(KH):
                    ps = ktpsum.tile([P, GB * S], F32, tag="ktp")
                    for k in range(KH):
                        nc.tensor.matmul(ps[:], UaT_sb[:, k, ts(m, P)], kts[k][:],
                                         start=(k == 0), stop=(k == KH - 1))
                    e_m = epool.tile([P, GB, S], BF16, tag="e")
                    for j in range(GB):
                        b = g * GB + j
                        nc.scalar.activation(e_m[:, j, :], ps[:, ts(j, S)], AF.Tanh,
                                             bias=qT[:, m, b:b + 1])
                    nc.tensor.matmul(sc_ps[:], va_t[:, m:m + 1], e_m[:],
                                     start=(m == 0), stop=(m == KH - 1))
                scb = opool.tile([1, GB * S], F32, tag="scb", name="scb")
                nc.vector.tensor_copy(scb[:], sc_ps[:])
                nc.sync.dma_start(out=sc_scratch_d[ts(g, GB * S)], in_=scb[:])

        # ============= Phase 6: softmax =============
        sc2 = gpool.tile([BC, S], F32, tag="sc2")
        nc.sync.dma_start(out=sc2[:], in_=sc_scratch_d[:].rearrange("(b s) -> b s", b=BC))
        mx = spool.tile([P, 1], F32, tag="mx")
        nc.vector.tensor_reduce(out=mx[:], in_=sc2[:], axis=mybir.AxisListType.X,
                                op=ALU.max)
        nmx = spool.tile([P, 1], F32, tag="nmx")
        nc.vector.tensor_scalar_mul(nmx[:], mx[:], -1.0)
        wsm = gpool.tile([BC, S], F32, tag="wsm")
        sume = spool.tile([P, 1], F32, tag="sume")
        nc.scalar.activation(wsm[:], sc2[:], AF.Exp, bias=nmx[:], accum_out=sume[:])
        rse = spool.tile([P, 1], F32, tag="rse")
        nc.vector.reciprocal(rse[:], sume[:])
        wsm_bf = gpool.tile([BC, S], BF16, tag="wsm_bf")
        nc.vector.tensor_scalar(wsm_bf[:], wsm[:], rse[:], None, ALU.mult)
        wT_sb = gpool.tile([S, BC], BF16, tag="wT")
        with tc.tile_pool(name="tp2", bufs=2, space="PSUM") as tp2:
            pe_transpose_to(wT_sb[:], wsm_bf[:], ident_bf, tp2, BF16)

        # ============= Phase 7: context =============
        ctxT = gpool.tile([P, KH, BC], F32, tag="sf", name="ctxT")
        with tc.tile_pool(name="cpsum", bufs=1, space="PSUM") as cpsum:
            cts = [cpsum.tile([P, BC], F32, tag=f"ct{m}", name=f"ct{m}") for m in range(KH)]
            for bb in range(NG):
                kn4 = knpool.tile([S, GB, H], BF16, tag="kn", name="kn4")
                nc.sync.dma_start(out=kn4[:],
                                  in_=keysN_d[ts(bb, GB), :, :].rearrange("g s h -> s g h"))
                for j in range(GB):
                    b = bb * GB + j
                    for m in range(KH):
                        nc.tensor.matmul(cts[m][:, b:b + 1], kn4[:, j, ts(m, P)],
                                         wT_sb[:, b:b + 1], start=True, stop=True)
            for m in range(KH):
                nc.vector.tensor_copy(ctxT[:, m, :], cts[m][:])

        ctxn = gpool.tile([BC, H], F32, tag="tg", name="ctxn")
        with tc.tile_pool(name="tp3", bufs=2, space="PSUM") as tp3:
            for m in range(KH):
                pe_transpose_to(ctxn[:, ts(m, P)], ctxT[:, m, :], ident_f, tp3, F32)

            # ============= Phase 8: LN3 =============
            nc.vector.tensor_tensor(out=ctxn[:], in0=hnorm_f[:], in1=ctxn[:],
                                    op=ALU.add)
            attnh_bf = gpool.tile([BC, H], BF16, tag="attnh_bf")
            layer_norm(ctxn[:], 2, None, attnh_bf[:])
            ahT = gpool.tile([P, KH, BC], BF16, tag="ahT")
            for k in range(KH):
                pe_transpose_to(ahT[:, k, :], attnh_bf[:, ts(k, P)], ident_bf, tp3, BF16)

        # ============= Phase 9: SwiGLU FFN =============
        NF = F // 512  # 8 chunks of the ffn dim
        with tc.tile_pool(name="fpsum", bufs=1, space="PSUM") as fpsum, \
             tc.tile_pool(name="upsum", bufs=2, space="PSUM") as upsum, \
             tc.tile_pool(name="tpsum", bufs=2, space="PSUM") as tpsum:
            ffd = [fpsum.tile([P, 512], F32, tag=f"ffd{h2}", name=f"ffd{h2}") for h2 in range(2)]
            for n in range(NF):
                u1 = upsum.tile([P, 512], F32, tag="u1")
                u3 = upsum.tile([P, 512], F32, tag="u3")
                for k in range(KH):
                    wt = wstream.tile([P, 512], BF16, tag="w")
                    nc.sync.dma_start(out=wt[:], in_=w1T_d[ts(k, P), ts(n, 512)])
                    nc.tensor.matmul(u1[:], ahT[:, k, :], wt[:],
                                     start=(k == 0), stop=(k == KH - 1))
                for k in range(KH):
                    wt = wstream.tile([P, 512], BF16, tag="w")
                    nc.sync.dma_start(out=wt[:], in_=w3T_d[ts(k, P), ts(n, 512)])
                    nc.tensor.matmul(u3[:], ahT[:, k, :], wt[:],
                                     start=(k == 0), stop=(k == KH - 1))
                sg = opool.tile([P, 512], F32, tag="sg")
                nc.scalar.activation(sg[:], u1[:], AF.Sigmoid, bias=zcol[:])
                v1 = opool.tile([P, 512], F32, tag="v1")
                nc.vector.tensor_tensor(out=v1[:], in0=u1[:], in1=sg[:], op=ALU.mult)
                v_bf = opool.tile([P, 512], BF16, tag="v_bf")
                nc.vector.tensor_tensor(out=v_bf[:], in0=v1[:], in1=u3[:], op=ALU.mult)
                for c in range(4):
                    f_idx = n * 4 + c
                    tp = tpsum.tile([P, P], BF16, tag="vtp")
                    nc.tensor.matmul(tp[:], v_bf[:, ts(c, P)], ident_bf[:],
                                     is_transpose=True)
                    vT = vtpool.tile([P, P], BF16, tag="vT")
                    nc.vector.tensor_copy(vT[:], tp[:])
                    for h2 in range(2):
                        wt = wstream.tile([P, 512], BF16, tag="w")
                        nc.sync.dma_start(out=wt[:], in_=w2T_d[ts(f_idx, P), ts(h2, 512)])
                        nc.tensor.matmul(ffd[h2][:], vT[:], wt[:],
                                         start=(f_idx == 0), stop=(f_idx == F // P - 1),
                                         skip_group_check=True)
            for h2 in range(2):
                ot = opool.tile([P, 512], F32, tag="ot")
                nc.vector.tensor_tensor(out=ot[:], in0=ffd[h2][:],
                                        in1=newh_t[:, ts(h2, 512)], op=ALU.add)
                nc.sync.dma_start(out=out_d[:, ts(h2, 512)], in_=ot[:])

    nc.compile()
    return nc


def prepare_inputs(inputs):
    """Host-side slicing / transposition / dtype casts. Returns per-core in_maps."""
    bf = ml_dtypes.bfloat16
    f32 = np.float32

    def c(a, dtype):
        return np.ascontiguousarray(a, dtype=dtype)

    x = np.asarray(inputs["x"], f32)
    h0 = np.asarray(inputs["h0"], f32)
    c0 = np.asarray(inputs["c0"], f32)
    keys = np.asarray(inputs["keys"], f32)

    shared = {
        "WihT": c(np.asarray(inputs["W_ih"]).T, bf),
        "WhhT": c(np.asarray(inputs["W_hh"]).T, bf),
        "biasg": c((np.asarray(inputs["b_ih"]) + np.asarray(inputs["b_hh"]))[None, :], bf),
        "WaT": c(np.asarray(inputs["Wa"]).T, bf),
        "Wab": c(np.asarray(inputs["Wa_b"])[None, :], bf),
        "UaT": c(np.asarray(inputs["Ua"]).T, bf),
        "Va8": c(np.asarray(inputs["Va"])[0].reshape(KH, P).T, bf),
        "w1T": c(np.asarray(inputs["w1"]).T, bf),
        "w3T": c(np.asarray(inputs["w3"]).T, bf),
        "w2T": c(np.asarray(inputs["w2"]).T, bf),
    }
    for i in (1, 2, 3):
        shared[f"ln{i}g"] = c(np.broadcast_to(np.asarray(inputs[f"ln{i}_g"]), (P, H)), f32)
        shared[f"ln{i}b"] = c(np.broadcast_to(np.asarray(inputs[f"ln{i}_b"]), (P, H)), f32)

    in_maps = []
    for core in range(NCORES):
        sl = slice(core * BC, (core + 1) * BC)
        kc = keys[sl]                                    # [BC, S, H]
        keysT = kc.reshape(NG, GB, S, H).transpose(0, 3, 1, 2)   # [NG, H, GB, S]
        m = dict(shared)
        m["x"] = c(x[sl], f32)
        m["h0T"] = c(h0[sl].T, bf)
        m["c0"] = c(c0[sl], f32)
        m["keysT"] = c(keysT, bf)
        m["keysN"] = c(kc, bf)
        in_maps.append(m)
    return in_maps


_PROGRAM = None


def kernel(**inputs):
    global _PROGRAM, LAST_RESULT
    if _PROGRAM is None:
        _PROGRAM = build_program()
    in_maps = prepare_inputs(inputs)
    res = run_bass_kernel_spmd(_PROGRAM, in_maps, list(range(NCORES)), trace=TRACE)
    LAST_RESULT = res
    outs = np.concatenate([np.asarray(r["out"]) for r in res.results], axis=0)
    newh = np.concatenate([np.asarray(r["new_h"]) for r in res.results], axis=0)
    cnew = np.concatenate([np.asarray(r["c_new"]) for r in res.results], axis=0)
    return outs.astype(np.float32), (newh.astype(np.float32), cnew.astype(np.float32))


# revision 10
# speedup vs baseline: 1.0620x; 1.0408x over previous
"""Trainium2 Bass kernel for nn_DecoderLayer (LSTM cell + Bahdanau attention + SwiGLU FFN).

Strategy: pure data-parallel over batch B=1024 across 8 cores (128 rows each, no
collectives). Host pre-transposes weights / packs keys so every matmul operand
streams from DRAM in its natural layout (contraction dim on partitions). All
matmul operands are bf16 (fp32 PSUM accumulation); LayerNorm / softmax / LSTM
elementwise math is fp32.
"""

import sys

for _p in ("/opt/trn_rl_repo",):
    if _p not in sys.path:
        sys.path.insert(0, _p)

from contextlib import ExitStack

import ml_dtypes
import numpy as np

import concourse.bass as bass
import concourse.tile as tile
from concourse import bacc, masks, mybir
from concourse.bass_utils import run_bass_kernel_spmd

dt = mybir.dt
ts = bass.ts
AF = mybir.ActivationFunctionType
ALU = mybir.AluOpType

B, S, H, F = 1024, 128, 1024, 4096
NCORES = 8
BC = B // NCORES          # 128 batch rows per core
P = 128                   # partitions
KH = H // P               # 8 contraction chunks over H
GB = 4                    # batch rows per attention group
NG = BC // GB             # 32 groups
H4 = 4 * H
EPS = 1e-5

BF16 = dt.bfloat16
F32 = dt.float32
F32R = dt.float32r
F8 = dt.float8e4

TRACE = False
LAST_RESULT = None


def _f32r(ap):
    return ap.bitcast(F32R)


def build_program():
    nc = bacc.Bacc("TRN2", target_bir_lowering=False, debug=False,
                   enable_asserts=True, num_devices=NCORES)

    def din(name, shape, dtype):
        return nc.dram_tensor(name, list(shape), dtype, kind="ExternalInput").ap()

    def dout(name, shape, dtype):
        return nc.dram_tensor(name, list(shape), dtype, kind="ExternalOutput").ap()

    x_d = din("x", (BC, H), F32)
    h0T_d = din("h0T", (H, BC), BF16)
    c0_d = din("c0", (BC, H), F32)
    keysT_d = din("keysT", (NG, H, GB, S), BF16)
    keysN_d = din("keysN", (BC, S, H), F8)
    WihT_d = din("WihT", (H, H4), BF16)
    WhhT_d = din("WhhT", (H, H4), BF16)
    biasg_d = din("biasg", (1, H4), BF16)
    WaT_d = din("WaT", (H, H), BF16)
    Wab_d = din("Wab", (1, H), BF16)
    UaT_d = din("UaT", (H, H), BF16)
    Va8_d = din("Va8", (P, KH), BF16)
    w1T_d = din("w1T", (H, F), BF16)
    w3T_d = din("w3T", (H, F), BF16)
    w2T_d = din("w2T", (F, H), BF16)
    lng_d = [din(f"ln{i}g", (P, H), F32) for i in (1, 2, 3)]
    lnb_d = [din(f"ln{i}b", (P, H), F32) for i in (1, 2, 3)]

    out_d = dout("out", (BC, H), F32)
    newh_d = dout("new_h", (BC, H), F32)
    cnew_d = dout("c_new", (BC, H), F32)

    sc_scratch_d = nc.dram_tensor("sc_scratch", [BC * S], F32).ap()

    with tile.TileContext(nc) as tc, ExitStack() as ctx:
        cpool = ctx.enter_context(tc.tile_pool(name="const", bufs=1))
        gpool = ctx.enter_context(tc.tile_pool(name="glob", bufs=1))
        spool = ctx.enter_context(tc.tile_pool(name="small", bufs=1))
        wres = ctx.enter_context(tc.tile_pool(name="wres", bufs=1))
        wstream = ctx.enter_context(tc.tile_pool(name="wstream", bufs=20))
        ktpool = ctx.enter_context(tc.tile_pool(name="ktpool", bufs=12))
        epool = ctx.enter_context(tc.tile_pool(name="epool", bufs=8))
        knpool = ctx.enter_context(tc.tile_pool(name="knpool", bufs=5))
        vtpool = ctx.enter_context(tc.tile_pool(name="vtpool", bufs=3))
        opool = ctx.enter_context(tc.tile_pool(name="opool", bufs=2))

        # ---- constants ----
        ident_bf = cpool.tile([P, P], BF16, tag="ident_bf")
        masks.make_identity(nc, ident_bf[:])
        ident_f = cpool.tile([P, P], F32, tag="ident_f")
        masks.make_identity(nc, ident_f[:])
        ones_f = cpool.tile([1, P], BF16, tag="ones_f")
        nc.vector.memset(ones_f[:], 1.0)
        zcol = cpool.tile([P, 1], F32, tag="zcol")
        nc.vector.memset(zcol[:], 0.0)
        ecol = cpool.tile([P, 1], F32, tag="ecol")
        nc.vector.memset(ecol[:], EPS)
        biasg_t = cpool.tile([1, H4], BF16, tag="biasg")
        nc.sync.dma_start(out=biasg_t[:], in_=biasg_d[:])
        wab_t = cpool.tile([1, H], BF16, tag="wab")
        nc.sync.dma_start(out=wab_t[:], in_=Wab_d[:])
        va_t = cpool.tile([P, KH], BF16, tag="va")
        nc.sync.dma_start(out=va_t[:], in_=Va8_d[:])

        lng_t = cpool.tile([P, H], F32, tag="lng")
        lnb_t = cpool.tile([P, H], F32, tag="lnb")

        # resident weight for the k projection: [p, kchunk, col]
        UaT_sb = wres.tile([P, KH, H], BF16, tag="UaT")
        nc.sync.dma_start(out=UaT_sb[:], in_=UaT_d[:].rearrange("(k p) m -> p k m", p=P))

        # ---- global activations ----
        x_t = gpool.tile([BC, H], F32, tag="x")
        nc.sync.dma_start(out=x_t[:], in_=x_d[:])
        c0_t = gpool.tile([BC, H], F32, tag="c0")
        nc.sync.dma_start(out=c0_t[:], in_=c0_d[:])
        h0T_t = gpool.tile([P, KH, BC], BF16, tag="h0T")
        nc.sync.dma_start(out=h0T_t[:], in_=h0T_d[:].rearrange("(k p) b -> p k b", p=P))

        def layer_norm(xin, ln_idx, out_f32, out_bf):
            """LN over free dim; writes fp32 result and optional bf16 copy."""
            nc.sync.dma_start(out=lng_t[:], in_=lng_d[ln_idx][:])
            nc.sync.dma_start(out=lnb_t[:], in_=lnb_d[ln_idx][:])
            tg = f"ln{ln_idx}"
            s1 = spool.tile([P, 1], F32, tag=tg + "s1")
            nc.vector.tensor_reduce(out=s1[:], in_=xin, axis=mybir.AxisListType.X,
                                    op=ALU.add)
            nm = spool.tile([P, 1], F32, tag=tg + "nm")
            nc.vector.tensor_scalar_mul(nm[:], s1[:], -1.0 / H)
            xc = gpool.tile([BC, H], F32, tag="ln_xc")
            nc.vector.tensor_scalar(xc[:], xin, nm[:], None, ALU.add)
            sq = gpool.tile([BC, H], F32, tag="scratch2", name="sq")
            ss = spool.tile([P, 1], F32, tag=tg + "ss")
            nc.scalar.activation(sq[:], xc[:], AF.Square, bias=zcol[:], accum_out=ss[:])
            sd = spool.tile([P, 1], F32, tag=tg + "sd")
            nc.scalar.activation(sd[:], ss[:], AF.Sqrt, bias=ecol[:], scale=1.0 / H)
            rstd = spool.tile([P, 1], F32, tag=tg + "rstd")
            nc.vector.reciprocal(rstd[:], sd[:])
            nc.vector.tensor_scalar(xc[:], xc[:], rstd[:], None, ALU.mult)
            nc.vector.tensor_tensor(out=xc[:], in0=xc[:], in1=lng_t[:], op=ALU.mult)
            if out_f32 is not None:
                nc.vector.tensor_tensor(out=out_f32, in0=xc[:], in1=lnb_t[:], op=ALU.add)
                if out_bf is not None:
                    nc.vector.tensor_copy(out_bf, out_f32)
            else:
                nc.vector.tensor_tensor(out=out_bf, in0=xc[:], in1=lnb_t[:], op=ALU.add)

        def pe_transpose_to(dst, src128, ident, tp_pool, dtype):
            t = tp_pool.tile([P, P], dtype, tag="tp")
            nc.tensor.matmul(t[:], src128, ident[:], is_transpose=True)
            nc.vector.tensor_copy(dst, t[:])

        # ================= Phase 1: LN1 + transpose =================
        xnorm_bf = gpool.tile([BC, H], BF16, tag="xnorm_bf")
        layer_norm(x_t[:], 0, None, xnorm_bf[:])

        xnT = gpool.tile([P, KH, BC], BF16, tag="xnT")
        with tc.tile_pool(name="tp1", bufs=2, space="PSUM") as tp1:
            for k in range(KH):
                pe_transpose_to(xnT[:, k, :], xnorm_bf[:, ts(k, P)], ident_bf, tp1, BF16)

            # ============= Phase 2: LSTM gates =============
            si = gpool.tile([BC, H], F32, tag="si")
            sf = gpool.tile([BC, H], F32, tag="sf")
            tg_ = gpool.tile([BC, H], F32, tag="tg")
            so = gpool.tile([BC, H], F32, tag="so")
            gate_sb = [si, sf, tg_, so]
            with tc.tile_pool(name="gpsum", bufs=4, space="PSUM") as gpsum:
                for half in range(2):
                    pss = [gpsum.tile([P, 512], F32, tag="g", name=f"gps{half}_{i}") for i in range(4)]
                    for k in range(KH):
                        for n in range(4):
                            nn = half * 4 + n
                            wt = wstream.tile([P, 512], BF16, tag="w")
                            nc.sync.dma_start(out=wt[:], in_=WihT_d[ts(k, P), ts(nn, 512)])
                            nc.tensor.matmul(pss[n][:], xnT[:, k, :], wt[:],
                                             start=(k == 0), stop=False)
                    for k in range(KH):
                        for n in range(4):
                            nn = half * 4 + n
                            wt = wstream.tile([P, 512], BF16, tag="w")
                            nc.sync.dma_start(out=wt[:], in_=WhhT_d[ts(k, P), ts(nn, 512)])
                            nc.tensor.matmul(pss[n][:], h0T_t[:, k, :], wt[:],
                                             start=False, stop=False)
                    for n in range(4):
                        nn = half * 4 + n
                        nc.tensor.matmul(pss[n][:], ones_f[:],
                                         biasg_t[:, ts(nn, 512)],
                                         start=False, stop=True)
                        gate = nn // 2   # 0:i 1:f 2:g 3:o
                        func = AF.Tanh if gate == 2 else AF.Sigmoid
                        nc.scalar.activation(gate_sb[gate][:, ts(nn % 2, 512)],
                                             pss[n][:], func, bias=zcol[:])

            # LSTM cell elementwise
            cn_t = gpool.tile([BC, H], F32, tag="cn")
            nc.vector.tensor_tensor(out=cn_t[:], in0=sf[:], in1=c0_t[:], op=ALU.mult)
            t2 = gpool.tile([BC, H], F32, tag="scratch2", name="t2")
            nc.vector.tensor_tensor(out=t2[:], in0=si[:], in1=tg_[:], op=ALU.mult)
            nc.vector.tensor_tensor(out=cn_t[:], in0=cn_t[:], in1=t2[:], op=ALU.add)
            nc.sync.dma_start(out=cnew_d[:], in_=cn_t[:])
            tcn = gpool.tile([BC, H], F32, tag="scratch2", name="tcn")
            nc.scalar.activation(tcn[:], cn_t[:], AF.Tanh, bias=zcol[:])
            newh_t = gpool.tile([BC, H], F32, tag="newh")
            nc.vector.tensor_tensor(out=newh_t[:], in0=so[:], in1=tcn[:], op=ALU.mult)
            nc.vector.tensor_tensor(out=newh_t[:], in0=x_t[:], in1=newh_t[:], op=ALU.add)
            nc.sync.dma_start(out=newh_d[:], in_=newh_t[:])

            # ============= Phase 3: LN2 + transpose =============
            hnorm_f = gpool.tile([BC, H], F32, tag="hnorm_f")
            hnorm_bf = gpool.tile([BC, H], BF16, tag="hnorm_bf")
            layer_norm(newh_t[:], 1, hnorm_f[:], hnorm_bf[:])
            hnT = gpool.tile([P, KH, BC], BF16, tag="hnT")
            for k in range(KH):
                pe_transpose_to(hnT[:, k, :], hnorm_bf[:, ts(k, P)], ident_bf, tp1, BF16)

        # ============= Phase 4: qT = Wa @ h_norm^T + Wa_b (transposed layout) ====
        qT = gpool.tile([P, KH, BC], F32, tag="si", name="qT")
        with tc.tile_pool(name="qpsum", bufs=2, space="PSUM") as qpsum:
            for m in range(KH):
                ps = qpsum.tile([P, BC], F32, tag="q")
                for k in range(KH):
                    wt = wstream.tile([P, P], BF16, tag="w", name="wa_t")
                    nc.sync.dma_start(out=wt[:], in_=WaT_d[ts(k, P), ts(m, P)])
                    nc.tensor.matmul(ps[:], wt[:], hnT[:, k, :],
                                     start=(k == 0), stop=False)
                nc.tensor.matmul(ps[:], wab_t[:, ts(m, P)], ones_f[:],
                                 start=False, stop=True)
                nc.vector.tensor_copy(qT[:, m, :], ps[:])

        # ============= Phase 5: attention scores =============
        with tc.tile_pool(name="ktpsum", bufs=5, space="PSUM") as ktpsum, \
             tc.tile_pool(name="scpsum", bufs=2, space="PSUM") as scpsum:
            for g in range(NG):
                kts = []
                for k in range(KH):
                    kt = ktpool.tile([P, GB, S], BF16, tag="kt")
                    nc.sync.dma_start(out=kt[:], in_=keysT_d[g, ts(k, P), :, :])
                    kts.append(kt)
                sc_ps = scpsum.tile([1, GB * S], F32, tag="sc")
                for m in range(KH):
                    ps = ktpsum.tile([P, GB * S], F32, tag="ktp")
                    for k in range(KH):
                        nc.tensor.matmul(ps[:], UaT_sb[:, k, ts(m, P)], kts[k][:],
                                         start=(k == 0), stop=(k == KH - 1))
                    e_m = epool.tile([P, GB, S], BF16, tag="e")
                    for j in range(GB):
                        b = g * GB + j
                        nc.scalar.activation(e_m[:, j, :], ps[:, ts(j, S)], AF.Tanh,
                                             bias=qT[:, m, b:b + 1])
                    nc.tensor.matmul(sc_ps[:], va_t[:, m:m + 1], e_m[:],
                                     start=(m == 0), stop=(m == KH - 1))
                scb = opool.tile([1, GB * S], F32, tag="scb", name="scb")
                nc.vector.tensor_copy(scb[:], sc_ps[:])
                nc.sync.dma_start(out=sc_scratch_d[ts(g, GB * S)], in_=scb[:])

        # ============= Phase 6: softmax =============
        sc2 = gpool.tile([BC, S], F32, tag="sc2")
        nc.sync.dma_start(out=sc2[:], in_=sc_scratch_d[:].rearrange("(b s) -> b s", b=BC))
        mx = spool.tile([P, 1], F32, tag="mx")
        nc.vector.tensor_reduce(out=mx[:], in_=sc2[:], axis=mybir.AxisListType.X,
                                op=ALU.max)
        nmx = spool.tile([P, 1], F32, tag="nmx")
        nc.vector.tensor_scalar_mul(nmx[:], mx[:], -1.0)
        wsm = gpool.tile([BC, S], F32, tag="wsm")
        sume = spool.tile([P, 1], F32, tag="sume")
        nc.scalar.activation(wsm[:], sc2[:], AF.Exp, bias=nmx[:], accum_out=sume[:])
        rse = spool.tile([P, 1], F32, tag="rse")
        nc.vector.reciprocal(rse[:], sume[:])
        wsm_bf = gpool.tile([BC, S], BF16, tag="wsm_bf")
        nc.vector.tensor_scalar(wsm_bf[:], wsm[:], rse[:], None, ALU.mult)
        wT_sb = gpool.tile([S, BC], BF16, tag="wT")
        with tc.tile_pool(name="tp2", bufs=2, space="PSUM") as tp2:
            pe_transpose_to(wT_sb[:], wsm_bf[:], ident_bf, tp2, BF16)

        # ============= Phase 7: context =============
        ctxT = gpool.tile([P, KH, BC], F32, tag="sf", name="ctxT")
        with tc.tile_pool(name="cpsum", bufs=1, space="PSUM") as cpsum:
            cts = [cpsum.tile([P, BC], F32, tag=f"ct{m}", name=f"ct{m}") for m in range(KH)]
            for bb in range(NG):
                kn4 = knpool.tile([S, GB, H], F8, tag="kn", name="kn4")
                nc.gpsimd.dma_start(out=kn4[:],
                                    in_=keysN_d[ts(bb, GB), :, :].rearrange("g s h -> s g h"))
                for j in range(GB):
                    b = bb * GB + j
                    for m in range(KH):
                        nc.tensor.matmul(cts[m][:, b:b + 1], kn4[:, j, ts(m, P)],
                                         wT_sb[:, b:b + 1], start=True, stop=True)
            for m in range(KH):
                nc.vector.tensor_copy(ctxT[:, m, :], cts[m][:])

        ctxn = gpool.tile([BC, H], F32, tag="tg", name="ctxn")
        with tc.tile_pool(name="tp3", bufs=2, space="PSUM") as tp3:
            for m in range(KH):
                pe_transpose_to(ctxn[:, ts(m, P)], ctxT[:, m, :], ident_f, tp3, F32)

            # ============= Phase 8: LN3 =============
            nc.vector.tensor_tensor(out=ctxn[:], in0=hnorm_f[:], in1=ctxn[:],
                                    op=ALU.add)
            attnh_bf = gpool.tile([BC, H], BF16, tag="attnh_bf")
            layer_norm(ctxn[:], 2, None, attnh_bf[:])
            ahT = gpool.tile([P, KH, BC], BF16, tag="ahT")
            for k in range(KH):
                pe_transpose_to(ahT[:, k, :], attnh_bf[:, ts(k, P)], ident_bf, tp3, BF16)

        # ============= Phase 9: SwiGLU FFN =============
        NF = F // 512  # 8 chunks of the ffn dim
        with tc.tile_pool(name="fpsum", bufs=1, space="PSUM") as fpsum, \
             tc.tile_pool(name="upsum", bufs=2, space="PSUM") as upsum, \
             tc.tile_pool(name="tpsum", bufs=2, space="PSUM") as tpsum:
            ffd = [fpsum.tile([P, 512], F32, tag=f"ffd{h2}", name=f"ffd{h2}") for h2 in range(2)]
            for n in range(NF):
                u1 = upsum.tile([P, 512], F32, tag="u1")
                u3 = upsum.tile([P, 512], F32, tag="u3")
                for k in range(KH):
                    wt = wstream.tile([P, 512], BF16, tag="w")
                    nc.gpsimd.dma_start(out=wt[:], in_=w1T_d[ts(k, P), ts(n, 512)])
                    nc.tensor.matmul(u1[:], ahT[:, k, :], wt[:],
                                     start=(k == 0), stop=(k == KH - 1))
                for k in range(KH):
                    wt = wstream.tile([P, 512], BF16, tag="w")
                    nc.gpsimd.dma_start(out=wt[:], in_=w3T_d[ts(k, P), ts(n, 512)])
                    nc.tensor.matmul(u3[:], ahT[:, k, :], wt[:],
                                     start=(k == 0), stop=(k == KH - 1))
                sg = opool.tile([P, 512], F32, tag="sg")
                nc.scalar.activation(sg[:], u1[:], AF.Sigmoid, bias=zcol[:])
                v1 = opool.tile([P, 512], F32, tag="v1")
                nc.vector.tensor_tensor(out=v1[:], in0=u1[:], in1=sg[:], op=ALU.mult)
                v_bf = opool.tile([P, 512], BF16, tag="v_bf")
                nc.vector.tensor_tensor(out=v_bf[:], in0=v1[:], in1=u3[:], op=ALU.mult)
                for c in range(4):
                    f_idx = n * 4 + c
                    tp = tpsum.tile([P, P], BF16, tag="vtp")
                    nc.tensor.matmul(tp[:], v_bf[:, ts(c, P)], ident_bf[:],
                                     is_transpose=True)
                    vT = vtpool.tile([P, P], BF16, tag="vT")
                    nc.vector.tensor_copy(vT[:], tp[:])
                    for h2 in range(2):
                        wt = wstream.tile([P, 512], BF16, tag="w")
                        nc.gpsimd.dma_start(out=wt[:], in_=w2T_d[ts(f_idx, P), ts(h2, 512)])
                        nc.tensor.matmul(ffd[h2][:], vT[:], wt[:],
                                         start=(f_idx == 0), stop=(f_idx == F // P - 1),
                                         skip_group_check=True)
            for h2 in range(2):
                ot = opool.tile([P, 512], F32, tag="ot")
                nc.vector.tensor_tensor(out=ot[:], in0=ffd[h2][:],
                                        in1=newh_t[:, ts(h2, 512)], op=ALU.add)
                nc.sync.dma_start(out=out_d[:, ts(h2, 512)], in_=ot[:])

    nc.compile()
    return nc


def prepare_inputs(inputs):
    """Host-side slicing / transposition / dtype casts. Returns per-core in_maps."""
    bf = ml_dtypes.bfloat16
    f32 = np.float32

    def c(a, dtype):
        return np.ascontiguousarray(a, dtype=dtype)

    x = np.asarray(inputs["x"], f32)
    h0 = np.asarray(inputs["h0"], f32)
    c0 = np.asarray(inputs["c0"], f32)
    keys = np.asarray(inputs["keys"], f32)

    shared = {
        "WihT": c(np.asarray(inputs["W_ih"]).T, bf),
        "WhhT": c(np.asarray(inputs["W_hh"]).T, bf),
        "biasg": c((np.asarray(inputs["b_ih"]) + np.asarray(inputs["b_hh"]))[None, :], bf),
        "WaT": c(np.asarray(inputs["Wa"]).T, bf),
        "Wab": c(np.asarray(inputs["Wa_b"])[None, :], bf),
        "UaT": c(np.asarray(inputs["Ua"]).T, bf),
        "Va8": c(np.asarray(inputs["Va"])[0].reshape(KH, P).T, bf),
        "w1T": c(np.asarray(inputs["w1"]).T, bf),
        "w3T": c(np.asarray(inputs["w3"]).T, bf),
        "w2T": c(np.asarray(inputs["w2"]).T, bf),
    }
    for i in (1, 2, 3):
        shared[f"ln{i}g"] = c(np.broadcast_to(np.asarray(inputs[f"ln{i}_g"]), (P, H)), f32)
        shared[f"ln{i}b"] = c(np.broadcast_to(np.asarray(inputs[f"ln{i}_b"]), (P, H)), f32)

    in_maps = []
    for core in range(NCORES):
        sl = slice(core * BC, (core + 1) * BC)
        kc = keys[sl]                                    # [BC, S, H]
        keysT = kc.reshape(NG, GB, S, H).transpose(0, 3, 1, 2)   # [NG, H, GB, S]
        m = dict(shared)
        m["x"] = c(x[sl], f32)
        m["h0T"] = c(h0[sl].T, bf)
        m["c0"] = c(c0[sl], f32)
        m["keysT"] = c(keysT, bf)
        m["keysN"] = c(kc, mybir.dt.np(F8))
        in_maps.append(m)
    return in_maps


_PROGRAM = None


def kernel(**inputs):
    global _PROGRAM, LAST_RESULT
    if _PROGRAM is None:
        _PROGRAM = build_program()
    in_maps = prepare_inputs(inputs)
    res = run_bass_kernel_spmd(_PROGRAM, in_maps, list(range(NCORES)), trace=TRACE)
    LAST_RESULT = res
    outs = np.concatenate([np.asarray(r["out"]) for r in res.results], axis=0)
    newh = np.concatenate([np.asarray(r["new_h"]) for r in res.results], axis=0)
    cnew = np.concatenate([np.asarray(r["c_new"]) for r in res.results], axis=0)
    return outs.astype(np.float32), (newh.astype(np.float32), cnew.astype(np.float32))


# revision 11
# speedup vs baseline: 1.4223x; 1.3392x over previous
"""Trainium2 Bass kernel for nn_DecoderLayer (LSTM cell + Bahdanau attention + SwiGLU FFN).

Strategy: pure data-parallel over batch B=1024 across 8 cores (128 rows each, no
collectives). Host pre-transposes weights / packs keys so every matmul operand
streams from DRAM in its natural layout (contraction dim on partitions). All
matmul operands are bf16 (fp32 PSUM accumulation); LayerNorm / softmax / LSTM
elementwise math is fp32.
"""

import sys

for _p in ("/opt/trn_rl_repo",):
    if _p not in sys.path:
        sys.path.insert(0, _p)

from contextlib import ExitStack

import ml_dtypes
import numpy as np

import concourse.bass as bass
import concourse.tile as tile
from concourse import bacc, masks, mybir
from concourse.bass_utils import run_bass_kernel_spmd

dt = mybir.dt
ts = bass.ts
AF = mybir.ActivationFunctionType
ALU = mybir.AluOpType

B, S, H, F = 1024, 128, 1024, 4096
NCORES = 8
BC = B // NCORES          # 128 batch rows per core
P = 128                   # partitions
KH = H // P               # 8 contraction chunks over H
GB = 4                    # batch rows per attention group
NG = BC // GB             # 32 groups
H4 = 4 * H
EPS = 1e-5

BF16 = dt.bfloat16
F32 = dt.float32
F32R = dt.float32r
F8 = dt.float8e4

TRACE = False
LAST_RESULT = None


def _f32r(ap):
    return ap.bitcast(F32R)


def build_program():
    nc = bacc.Bacc("TRN2", target_bir_lowering=False, debug=False,
                   enable_asserts=True, num_devices=NCORES)

    def din(name, shape, dtype):
        return nc.dram_tensor(name, list(shape), dtype, kind="ExternalInput").ap()

    def dout(name, shape, dtype):
        return nc.dram_tensor(name, list(shape), dtype, kind="ExternalOutput").ap()

    x_d = din("x", (BC, H), F32)
    h0T_d = din("h0T", (H, BC), BF16)
    c0_d = din("c0", (BC, H), F32)
    keysT_d = din("keysT", (NG, H, GB, S), F8)
    keysN_d = din("keysN", (BC, S, H), F8)
    WihT_d = din("WihT", (H, H4), BF16)
    WhhT_d = din("WhhT", (H, H4), BF16)
    biasg_d = din("biasg", (1, H4), BF16)
    WaT_d = din("WaT", (H, H), BF16)
    Wab_d = din("Wab", (1, H), BF16)
    UaT_d = din("UaT", (H, H), F8)
    Va8_d = din("Va8", (P, KH), BF16)
    w1T_d = din("w1T", (H, F), BF16)
    w3T_d = din("w3T", (H, F), BF16)
    w2T_d = din("w2T", (F, H), BF16)
    lng_d = [din(f"ln{i}g", (P, H), F32) for i in (1, 2, 3)]
    lnb_d = [din(f"ln{i}b", (P, H), F32) for i in (1, 2, 3)]

    out_d = dout("out", (BC, H), F32)
    newh_d = dout("new_h", (BC, H), F32)
    cnew_d = dout("c_new", (BC, H), F32)

    sc_scratch_d = nc.dram_tensor("sc_scratch", [BC * S], F32).ap()

    with tile.TileContext(nc) as tc, ExitStack() as ctx:
        cpool = ctx.enter_context(tc.tile_pool(name="const", bufs=1))
        gpool = ctx.enter_context(tc.tile_pool(name="glob", bufs=1))
        spool = ctx.enter_context(tc.tile_pool(name="small", bufs=1))
        wres = ctx.enter_context(tc.tile_pool(name="wres", bufs=1))
        wstream = ctx.enter_context(tc.tile_pool(name="wstream", bufs=20))
        ktpool = ctx.enter_context(tc.tile_pool(name="ktpool", bufs=12))
        epool = ctx.enter_context(tc.tile_pool(name="epool", bufs=8))
        knpool = ctx.enter_context(tc.tile_pool(name="knpool", bufs=5))
        vtpool = ctx.enter_context(tc.tile_pool(name="vtpool", bufs=3))
        opool = ctx.enter_context(tc.tile_pool(name="opool", bufs=2))

        # ---- constants ----
        ident_bf = cpool.tile([P, P], BF16, tag="ident_bf")
        masks.make_identity(nc, ident_bf[:])
        ident_f = cpool.tile([P, P], F32, tag="ident_f")
        masks.make_identity(nc, ident_f[:])
        ones_f = cpool.tile([1, P], BF16, tag="ones_f")
        nc.vector.memset(ones_f[:], 1.0)
        zcol = cpool.tile([P, 1], F32, tag="zcol")
        nc.vector.memset(zcol[:], 0.0)
        ecol = cpool.tile([P, 1], F32, tag="ecol")
        nc.vector.memset(ecol[:], EPS)
        biasg_t = cpool.tile([1, H4], BF16, tag="biasg")
        nc.sync.dma_start(out=biasg_t[:], in_=biasg_d[:])
        wab_t = cpool.tile([1, H], BF16, tag="wab")
        nc.sync.dma_start(out=wab_t[:], in_=Wab_d[:])
        va_t = cpool.tile([P, KH], BF16, tag="va")
        nc.sync.dma_start(out=va_t[:], in_=Va8_d[:])

        lng_t = cpool.tile([P, H], F32, tag="lng")
        lnb_t = cpool.tile([P, H], F32, tag="lnb")

        # resident weight for the k projection: [p, kchunk, col]
        UaT_sb = wres.tile([P, KH, H], F8, tag="UaT")
        nc.sync.dma_start(out=UaT_sb[:], in_=UaT_d[:].rearrange("(k p) m -> p k m", p=P))

        # ---- global activations ----
        x_t = gpool.tile([BC, H], F32, tag="x")
        nc.sync.dma_start(out=x_t[:], in_=x_d[:])
        c0_t = gpool.tile([BC, H], F32, tag="c0")
        nc.sync.dma_start(out=c0_t[:], in_=c0_d[:])
        h0T_t = gpool.tile([P, KH, BC], BF16, tag="h0T")
        nc.sync.dma_start(out=h0T_t[:], in_=h0T_d[:].rearrange("(k p) b -> p k b", p=P))

        def layer_norm(xin, ln_idx, out_f32, out_bf):
            """LN over free dim; writes fp32 result and optional bf16 copy."""
            nc.sync.dma_start(out=lng_t[:], in_=lng_d[ln_idx][:])
            nc.sync.dma_start(out=lnb_t[:], in_=lnb_d[ln_idx][:])
            tg = f"ln{ln_idx}"
            s1 = spool.tile([P, 1], F32, tag=tg + "s1")
            nc.vector.tensor_reduce(out=s1[:], in_=xin, axis=mybir.AxisListType.X,
                                    op=ALU.add)
            nm = spool.tile([P, 1], F32, tag=tg + "nm")
            nc.vector.tensor_scalar_mul(nm[:], s1[:], -1.0 / H)
            xc = gpool.tile([BC, H], F32, tag="ln_xc")
            nc.vector.tensor_scalar(xc[:], xin, nm[:], None, ALU.add)
            sq = gpool.tile([BC, H], F32, tag="scratch2", name="sq")
            ss = spool.tile([P, 1], F32, tag=tg + "ss")
            nc.scalar.activation(sq[:], xc[:], AF.Square, bias=zcol[:], accum_out=ss[:])
            sd = spool.tile([P, 1], F32, tag=tg + "sd")
            nc.scalar.activation(sd[:], ss[:], AF.Sqrt, bias=ecol[:], scale=1.0 / H)
            rstd = spool.tile([P, 1], F32, tag=tg + "rstd")
            nc.vector.reciprocal(rstd[:], sd[:])
            nc.vector.tensor_scalar(xc[:], xc[:], rstd[:], None, ALU.mult)
            nc.vector.tensor_tensor(out=xc[:], in0=xc[:], in1=lng_t[:], op=ALU.mult)
            if out_f32 is not None:
                nc.vector.tensor_tensor(out=out_f32, in0=xc[:], in1=lnb_t[:], op=ALU.add)
                if out_bf is not None:
                    nc.vector.tensor_copy(out_bf, out_f32)
            else:
                nc.vector.tensor_tensor(out=out_bf, in0=xc[:], in1=lnb_t[:], op=ALU.add)

        def pe_transpose_to(dst, src128, ident, tp_pool, dtype):
            t = tp_pool.tile([P, P], dtype, tag="tp")
            nc.tensor.matmul(t[:], src128, ident[:], is_transpose=True)
            nc.vector.tensor_copy(dst, t[:])

        # ================= Phase 1: LN1 + transpose =================
        xnorm_bf = gpool.tile([BC, H], BF16, tag="xnorm_bf")
        layer_norm(x_t[:], 0, None, xnorm_bf[:])

        xnT = gpool.tile([P, KH, BC], BF16, tag="xnT")
        with tc.tile_pool(name="tp1", bufs=2, space="PSUM") as tp1:
            for k in range(KH):
                pe_transpose_to(xnT[:, k, :], xnorm_bf[:, ts(k, P)], ident_bf, tp1, BF16)

            # ============= Phase 2: LSTM gates =============
            si = gpool.tile([BC, H], F32, tag="si")
            sf = gpool.tile([BC, H], F32, tag="sf")
            tg_ = gpool.tile([BC, H], F32, tag="tg")
            so = gpool.tile([BC, H], F32, tag="so")
            gate_sb = [si, sf, tg_, so]
            with tc.tile_pool(name="gpsum", bufs=4, space="PSUM") as gpsum:
                for half in range(2):
                    pss = [gpsum.tile([P, 512], F32, tag="g", name=f"gps{half}_{i}") for i in range(4)]
                    for k in range(KH):
                        for n in range(4):
                            nn = half * 4 + n
                            wt = wstream.tile([P, 512], BF16, tag="w")
                            nc.sync.dma_start(out=wt[:], in_=WihT_d[ts(k, P), ts(nn, 512)])
                            nc.tensor.matmul(pss[n][:], xnT[:, k, :], wt[:],
                                             start=(k == 0), stop=False)
                    for k in range(KH):
                        for n in range(4):
                            nn = half * 4 + n
                            wt = wstream.tile([P, 512], BF16, tag="w")
                            nc.sync.dma_start(out=wt[:], in_=WhhT_d[ts(k, P), ts(nn, 512)])
                            nc.tensor.matmul(pss[n][:], h0T_t[:, k, :], wt[:],
                                             start=False, stop=False)
                    for n in range(4):
                        nn = half * 4 + n
                        nc.tensor.matmul(pss[n][:], ones_f[:],
                                         biasg_t[:, ts(nn, 512)],
                                         start=False, stop=True)
                        gate = nn // 2   # 0:i 1:f 2:g 3:o
                        func = AF.Tanh if gate == 2 else AF.Sigmoid
                        nc.scalar.activation(gate_sb[gate][:, ts(nn % 2, 512)],
                                             pss[n][:], func, bias=zcol[:])

            # LSTM cell elementwise
            cn_t = gpool.tile([BC, H], F32, tag="cn")
            nc.vector.tensor_tensor(out=cn_t[:], in0=sf[:], in1=c0_t[:], op=ALU.mult)
            t2 = gpool.tile([BC, H], F32, tag="scratch2", name="t2")
            nc.vector.tensor_tensor(out=t2[:], in0=si[:], in1=tg_[:], op=ALU.mult)
            nc.vector.tensor_tensor(out=cn_t[:], in0=cn_t[:], in1=t2[:], op=ALU.add)
            nc.sync.dma_start(out=cnew_d[:], in_=cn_t[:])
            tcn = gpool.tile([BC, H], F32, tag="scratch2", name="tcn")
            nc.scalar.activation(tcn[:], cn_t[:], AF.Tanh, bias=zcol[:])
            newh_t = gpool.tile([BC, H], F32, tag="newh")
            nc.vector.tensor_tensor(out=newh_t[:], in0=so[:], in1=tcn[:], op=ALU.mult)
            nc.vector.tensor_tensor(out=newh_t[:], in0=x_t[:], in1=newh_t[:], op=ALU.add)
            nc.sync.dma_start(out=newh_d[:], in_=newh_t[:])

            # ============= Phase 3: LN2 + transpose =============
            hnorm_f = gpool.tile([BC, H], F32, tag="hnorm_f")
            hnorm_bf = gpool.tile([BC, H], BF16, tag="hnorm_bf")
            layer_norm(newh_t[:], 1, hnorm_f[:], hnorm_bf[:])
            hnT = gpool.tile([P, KH, BC], BF16, tag="hnT")
            for k in range(KH):
                pe_transpose_to(hnT[:, k, :], hnorm_bf[:, ts(k, P)], ident_bf, tp1, BF16)

        # ============= Phase 4: qT = Wa @ h_norm^T + Wa_b (transposed layout) ====
        qT = gpool.tile([P, KH, BC], F32, tag="si", name="qT")
        with tc.tile_pool(name="qpsum", bufs=2, space="PSUM") as qpsum:
            for m in range(KH):
                ps = qpsum.tile([P, BC], F32, tag="q")
                for k in range(KH):
                    wt = wstream.tile([P, P], BF16, tag="w", name="wa_t")
                    nc.sync.dma_start(out=wt[:], in_=WaT_d[ts(k, P), ts(m, P)])
                    nc.tensor.matmul(ps[:], wt[:], hnT[:, k, :],
                                     start=(k == 0), stop=False)
                nc.tensor.matmul(ps[:], wab_t[:, ts(m, P)], ones_f[:],
                                 start=False, stop=True)
                nc.vector.tensor_copy(qT[:, m, :], ps[:])

        # ============= Phase 5: attention scores =============
        with tc.tile_pool(name="ktpsum", bufs=5, space="PSUM") as ktpsum, \
             tc.tile_pool(name="scpsum", bufs=2, space="PSUM") as scpsum:
            for g in range(NG):
                kts = []
                for k2 in range(KH // 2):
                    kt = ktpool.tile([P, 2, GB, S], F8, tag="kt", name="kt")
                    nc.sync.dma_start(
                        out=kt[:],
                        in_=keysT_d[g, ts(k2, 2 * P), :, :].rearrange(
                            "(two p) g s -> p two g s", two=2))
                    kts.append(kt)
                sc_ps = scpsum.tile([1, GB * S], F32, tag="sc")
                for m in range(KH):
                    ps = ktpsum.tile([P, GB * S], F32, tag="ktp")
                    for k2 in range(KH // 2):
                        nc.tensor.matmul(ps[:], UaT_sb[:, ts(k2, 2), ts(m, P)],
                                         kts[k2][:],
                                         start=(k2 == 0), stop=(k2 == KH // 2 - 1),
                                         perf_mode=mybir.MatmulPerfMode.DoubleRow)
                    e_m = epool.tile([P, GB, S], BF16, tag="e")
                    for j in range(GB):
                        b = g * GB + j
                        nc.scalar.activation(e_m[:, j, :], ps[:, ts(j, S)], AF.Tanh,
                                             bias=qT[:, m, b:b + 1])
                    nc.tensor.matmul(sc_ps[:], va_t[:, m:m + 1], e_m[:],
                                     start=(m == 0), stop=(m == KH - 1))
                scb = opool.tile([1, GB * S], F32, tag="scb", name="scb")
                nc.vector.tensor_copy(scb[:], sc_ps[:])
                nc.sync.dma_start(out=sc_scratch_d[ts(g, GB * S)], in_=scb[:])

        # ============= Phase 6: softmax =============
        sc2 = gpool.tile([BC, S], F32, tag="sc2")
        nc.sync.dma_start(out=sc2[:], in_=sc_scratch_d[:].rearrange("(b s) -> b s", b=BC))
        mx = spool.tile([P, 1], F32, tag="mx")
        nc.vector.tensor_reduce(out=mx[:], in_=sc2[:], axis=mybir.AxisListType.X,
                                op=ALU.max)
        nmx = spool.tile([P, 1], F32, tag="nmx")
        nc.vector.tensor_scalar_mul(nmx[:], mx[:], -1.0)
        wsm = gpool.tile([BC, S], F32, tag="wsm")
        sume = spool.tile([P, 1], F32, tag="sume")
        nc.scalar.activation(wsm[:], sc2[:], AF.Exp, bias=nmx[:], accum_out=sume[:])
        rse = spool.tile([P, 1], F32, tag="rse")
        nc.vector.reciprocal(rse[:], sume[:])
        wsm_bf = gpool.tile([BC, S], BF16, tag="wsm_bf")
        nc.vector.tensor_scalar(wsm_bf[:], wsm[:], rse[:], None, ALU.mult)
        wT_sb = gpool.tile([S, BC], BF16, tag="wT")
        with tc.tile_pool(name="tp2", bufs=2, space="PSUM") as tp2:
            pe_transpose_to(wT_sb[:], wsm_bf[:], ident_bf, tp2, BF16)

        # ============= Phase 7: context =============
        ctxT = gpool.tile([P, KH, BC], F32, tag="sf", name="ctxT")
        with tc.tile_pool(name="cpsum", bufs=1, space="PSUM") as cpsum:
            cts = [cpsum.tile([P, BC], F32, tag=f"ct{m}", name=f"ct{m}") for m in range(KH)]
            for bb in range(NG):
                kn4 = knpool.tile([S, GB, H], F8, tag="kn", name="kn4")
                nc.gpsimd.dma_start(out=kn4[:],
                                    in_=keysN_d[ts(bb, GB), :, :].rearrange("g s h -> s g h"))
                for j in range(GB):
                    b = bb * GB + j
                    for m in range(KH):
                        nc.tensor.matmul(cts[m][:, b:b + 1], kn4[:, j, ts(m, P)],
                                         wT_sb[:, b:b + 1], start=True, stop=True)
            for m in range(KH):
                nc.vector.tensor_copy(ctxT[:, m, :], cts[m][:])

        ctxn = gpool.tile([BC, H], F32, tag="tg", name="ctxn")
        with tc.tile_pool(name="tp3", bufs=2, space="PSUM") as tp3:
            for m in range(KH):
                pe_transpose_to(ctxn[:, ts(m, P)], ctxT[:, m, :], ident_f, tp3, F32)

            # ============= Phase 8: LN3 =============
            nc.vector.tensor_tensor(out=ctxn[:], in0=hnorm_f[:], in1=ctxn[:],
                                    op=ALU.add)
            attnh_bf = gpool.tile([BC, H], BF16, tag="attnh_bf")
            layer_norm(ctxn[:], 2, None, attnh_bf[:])
            ahT = gpool.tile([P, KH, BC], BF16, tag="ahT")
            for k in range(KH):
                pe_transpose_to(ahT[:, k, :], attnh_bf[:, ts(k, P)], ident_bf, tp3, BF16)

        # ============= Phase 9: SwiGLU FFN =============
        NF = F // 512  # 8 chunks of the ffn dim
        with tc.tile_pool(name="fpsum", bufs=1, space="PSUM") as fpsum, \
             tc.tile_pool(name="upsum", bufs=2, space="PSUM") as upsum, \
             tc.tile_pool(name="tpsum", bufs=2, space="PSUM") as tpsum:
            ffd = [fpsum.tile([P, 512], F32, tag=f"ffd{h2}", name=f"ffd{h2}") for h2 in range(2)]
            for n in range(NF):
                u1 = upsum.tile([P, 512], F32, tag="u1")
                u3 = upsum.tile([P, 512], F32, tag="u3")
                for k in range(KH):
                    wt = wstream.tile([P, 512], BF16, tag="w")
                    nc.gpsimd.dma_start(out=wt[:], in_=w1T_d[ts(k, P), ts(n, 512)])
                    nc.tensor.matmul(u1[:], ahT[:, k, :], wt[:],
                                     start=(k == 0), stop=(k == KH - 1))
                for k in range(KH):
                    wt = wstream.tile([P, 512], BF16, tag="w")
                    nc.gpsimd.dma_start(out=wt[:], in_=w3T_d[ts(k, P), ts(n, 512)])
                    nc.tensor.matmul(u3[:], ahT[:, k, :], wt[:],
                                     start=(k == 0), stop=(k == KH - 1))
                sg = opool.tile([P, 512], F32, tag="sg")
                nc.scalar.activation(sg[:], u1[:], AF.Sigmoid, bias=zcol[:])
                v1 = opool.tile([P, 512], F32, tag="v1")
                nc.vector.tensor_tensor(out=v1[:], in0=u1[:], in1=sg[:], op=ALU.mult)
                v_bf = opool.tile([P, 512], BF16, tag="v_bf")
                nc.vector.tensor_tensor(out=v_bf[:], in0=v1[:], in1=u3[:], op=ALU.mult)
                for c in range(4):
                    f_idx = n * 4 + c
                    tp = tpsum.tile([P, P], BF16, tag="vtp")
                    nc.tensor.matmul(tp[:], v_bf[:, ts(c, P)], ident_bf[:],
                                     is_transpose=True)
                    vT = vtpool.tile([P, P], BF16, tag="vT")
                    nc.vector.tensor_copy(vT[:], tp[:])
                    for h2 in range(2):
                        wt = wstream.tile([P, 512], BF16, tag="w")
                        nc.gpsimd.dma_start(out=wt[:], in_=w2T_d[ts(f_idx, P), ts(h2, 512)])
                        nc.tensor.matmul(ffd[h2][:], vT[:], wt[:],
                                         start=(f_idx == 0), stop=(f_idx == F // P - 1),
                                         skip_group_check=True)
            for h2 in range(2):
                ot = opool.tile([P, 512], F32, tag="ot")
                nc.vector.tensor_tensor(out=ot[:], in0=ffd[h2][:],
                                        in1=newh_t[:, ts(h2, 512)], op=ALU.add)
                nc.sync.dma_start(out=out_d[:, ts(h2, 512)], in_=ot[:])

    nc.compile()
    return nc


def prepare_inputs(inputs):
    """Host-side slicing / transposition / dtype casts. Returns per-core in_maps."""
    bf = ml_dtypes.bfloat16
    f32 = np.float32

    def c(a, dtype):
        return np.ascontiguousarray(a, dtype=dtype)

    x = np.asarray(inputs["x"], f32)
    h0 = np.asarray(inputs["h0"], f32)
    c0 = np.asarray(inputs["c0"], f32)
    keys = np.asarray(inputs["keys"], f32)

    shared = {
        "WihT": c(np.asarray(inputs["W_ih"]).T, bf),
        "WhhT": c(np.asarray(inputs["W_hh"]).T, bf),
        "biasg": c((np.asarray(inputs["b_ih"]) + np.asarray(inputs["b_hh"]))[None, :], bf),
        "WaT": c(np.asarray(inputs["Wa"]).T, bf),
        "Wab": c(np.asarray(inputs["Wa_b"])[None, :], bf),
        "UaT": c(np.asarray(inputs["Ua"]).T, mybir.dt.np(F8)),
        "Va8": c(np.asarray(inputs["Va"])[0].reshape(KH, P).T, bf),
        "w1T": c(np.asarray(inputs["w1"]).T, bf),
        "w3T": c(np.asarray(inputs["w3"]).T, bf),
        "w2T": c(np.asarray(inputs["w2"]).T, bf),
    }
    for i in (1, 2, 3):
        shared[f"ln{i}g"] = c(np.broadcast_to(np.asarray(inputs[f"ln{i}_g"]), (P, H)), f32)
        shared[f"ln{i}b"] = c(np.broadcast_to(np.asarray(inputs[f"ln{i}_b"]), (P, H)), f32)

    in_maps = []
    for core in range(NCORES):
        sl = slice(core * BC, (core + 1) * BC)
        kc = keys[sl]                                    # [BC, S, H]
        keysT = kc.reshape(NG, GB, S, H).transpose(0, 3, 1, 2)   # [NG, H, GB, S]
        m = dict(shared)
        m["x"] = c(x[sl], f32)
        m["h0T"] = c(h0[sl].T, bf)
        m["c0"] = c(c0[sl], f32)
        m["keysT"] = c(keysT, mybir.dt.np(F8))
        m["keysN"] = c(kc, mybir.dt.np(F8))
        in_maps.append(m)
    return in_maps


_PROGRAM = None


def kernel(**inputs):
    global _PROGRAM, LAST_RESULT
    if _PROGRAM is None:
        _PROGRAM = build_program()
    in_maps = prepare_inputs(inputs)
    res = run_bass_kernel_spmd(_PROGRAM, in_maps, list(range(NCORES)), trace=TRACE)
    LAST_RESULT = res
    outs = np.concatenate([np.asarray(r["out"]) for r in res.results], axis=0)
    newh = np.concatenate([np.asarray(r["new_h"]) for r in res.results], axis=0)
    cnew = np.concatenate([np.asarray(r["c_new"]) for r in res.results], axis=0)
    return outs.astype(np.float32), (newh.astype(np.float32), cnew.astype(np.float32))


# revision 13
# speedup vs baseline: 1.5785x; 1.1098x over previous
"""Trainium2 Bass kernel for nn_DecoderLayer (LSTM cell + Bahdanau attention + SwiGLU FFN).

Strategy: pure data-parallel over batch B=1024 across 8 cores (128 rows each, no
collectives). Host pre-transposes weights / packs keys so every matmul operand
streams from DRAM in its natural layout (contraction dim on partitions). All
matmul operands are bf16 (fp32 PSUM accumulation); LayerNorm / softmax / LSTM
elementwise math is fp32.
"""

import sys

for _p in ("/opt/trn_rl_repo",):
    if _p not in sys.path:
        sys.path.insert(0, _p)

from contextlib import ExitStack

import ml_dtypes
import numpy as np

import concourse.bass as bass
import concourse.tile as tile
from concourse import bacc, masks, mybir
from concourse.bass_utils import run_bass_kernel_spmd

dt = mybir.dt
ts = bass.ts
AF = mybir.ActivationFunctionType
ALU = mybir.AluOpType

B, S, H, F = 1024, 128, 1024, 4096
NCORES = 8
BC = B // NCORES          # 128 batch rows per core
P = 128                   # partitions
KH = H // P               # 8 contraction chunks over H
GB = 4                    # batch rows per attention group
NG = BC // GB             # 32 groups
H4 = 4 * H
EPS = 1e-5

BF16 = dt.bfloat16
F32 = dt.float32
F32R = dt.float32r
F8 = dt.float8e4

TRACE = False
LAST_RESULT = None


def _f32r(ap):
    return ap.bitcast(F32R)


def build_program():
    nc = bacc.Bacc("TRN2", target_bir_lowering=False, debug=False,
                   enable_asserts=True, num_devices=NCORES)

    def din(name, shape, dtype):
        return nc.dram_tensor(name, list(shape), dtype, kind="ExternalInput").ap()

    def dout(name, shape, dtype):
        return nc.dram_tensor(name, list(shape), dtype, kind="ExternalOutput").ap()

    x_d = din("x", (BC, H), F32)
    h0T_d = din("h0T", (H, BC), BF16)
    c0_d = din("c0", (BC, H), F32)
    keysT_d = din("keysT", (NG, H, GB, S), F8)
    keysN_d = din("keysN", (BC, S, H), F8)
    WihT_d = din("WihT", (H, H4), BF16)
    WhhT_d = din("WhhT", (H, H4), BF16)
    biasg_d = din("biasg", (1, H4), BF16)
    WaT_d = din("WaT", (H, H), BF16)
    Wab_d = din("Wab", (1, H), BF16)
    UaT_d = din("UaT", (H, H), F8)
    Va8_d = din("Va8", (P, KH, 16), F8)
    w1T_d = din("w1T", (H, F), BF16)
    w3T_d = din("w3T", (H, F), BF16)
    w2T_d = din("w2T", (F, H), BF16)
    lng_d = [din(f"ln{i}g", (P, H), F32) for i in (1, 2, 3)]
    lnb_d = [din(f"ln{i}b", (P, H), F32) for i in (1, 2, 3)]

    out_d = dout("out", (BC, H), F32)
    newh_d = dout("new_h", (BC, H), F32)
    cnew_d = dout("c_new", (BC, H), F32)

    sc_scratch_d = nc.dram_tensor("sc_scratch", [BC * S], F32).ap()

    with tile.TileContext(nc) as tc, ExitStack() as ctx:
        cpool = ctx.enter_context(tc.tile_pool(name="const", bufs=1))
        gpool = ctx.enter_context(tc.tile_pool(name="glob", bufs=1))
        spool = ctx.enter_context(tc.tile_pool(name="small", bufs=1))
        wres = ctx.enter_context(tc.tile_pool(name="wres", bufs=1))
        wstream = ctx.enter_context(tc.tile_pool(name="wstream", bufs=28))
        ktpool = ctx.enter_context(tc.tile_pool(name="ktpool", bufs=12))
        epool = ctx.enter_context(tc.tile_pool(name="epool", bufs=8))
        knpool = ctx.enter_context(tc.tile_pool(name="knpool", bufs=8))
        vtpool = ctx.enter_context(tc.tile_pool(name="vtpool", bufs=3))
        opool = ctx.enter_context(tc.tile_pool(name="opool", bufs=2))

        # ---- constants ----
        ident_bf = cpool.tile([P, P], BF16, tag="ident_bf")
        masks.make_identity(nc, ident_bf[:])
        ident_f = cpool.tile([P, P], F32, tag="ident_f")
        masks.make_identity(nc, ident_f[:])
        ones_f = cpool.tile([1, P], BF16, tag="ones_f")
        nc.vector.memset(ones_f[:], 1.0)
        zcol = cpool.tile([P, 1], F32, tag="zcol")
        nc.vector.memset(zcol[:], 0.0)
        ecol = cpool.tile([P, 1], F32, tag="ecol")
        nc.vector.memset(ecol[:], EPS)
        biasg_t = cpool.tile([1, H4], BF16, tag="biasg")
        nc.sync.dma_start(out=biasg_t[:], in_=biasg_d[:])
        wab_t = cpool.tile([1, H], BF16, tag="wab")
        nc.sync.dma_start(out=wab_t[:], in_=Wab_d[:])
        va_t = cpool.tile([P, KH, 16], F8, tag="va")
        nc.sync.dma_start(out=va_t[:], in_=Va8_d[:])

        lng_t = cpool.tile([P, H], F32, tag="lng")
        lnb_t = cpool.tile([P, H], F32, tag="lnb")

        # resident weight for the k projection: [p, kchunk, col]
        UaT_sb = wres.tile([P, KH, H], F8, tag="UaT")
        nc.sync.dma_start(out=UaT_sb[:], in_=UaT_d[:].rearrange("(k p) m -> p k m", p=P))

        # ---- global activations ----
        x_t = gpool.tile([BC, H], F32, tag="x")
        nc.sync.dma_start(out=x_t[:], in_=x_d[:])
        c0_t = gpool.tile([BC, H], F32, tag="c0")
        nc.sync.dma_start(out=c0_t[:], in_=c0_d[:])
        h0T_t = gpool.tile([P, KH, BC], BF16, tag="h0T")
        nc.sync.dma_start(out=h0T_t[:], in_=h0T_d[:].rearrange("(k p) b -> p k b", p=P))

        def layer_norm(xin, ln_idx, out_f32, out_bf):
            """LN over free dim; writes fp32 result and optional bf16 copy."""
            nc.sync.dma_start(out=lng_t[:], in_=lng_d[ln_idx][:])
            nc.sync.dma_start(out=lnb_t[:], in_=lnb_d[ln_idx][:])
            tg = f"ln{ln_idx}"
            s1 = spool.tile([P, 1], F32, tag=tg + "s1")
            nc.vector.tensor_reduce(out=s1[:], in_=xin, axis=mybir.AxisListType.X,
                                    op=ALU.add)
            nm = spool.tile([P, 1], F32, tag=tg + "nm")
            nc.vector.tensor_scalar_mul(nm[:], s1[:], -1.0 / H)
            xc = gpool.tile([BC, H], F32, tag="ln_xc")
            nc.vector.tensor_scalar(xc[:], xin, nm[:], None, ALU.add)
            sq = gpool.tile([BC, H], F32, tag="scratch2", name="sq")
            ss = spool.tile([P, 1], F32, tag=tg + "ss")
            nc.scalar.activation(sq[:], xc[:], AF.Square, bias=zcol[:], accum_out=ss[:])
            sd = spool.tile([P, 1], F32, tag=tg + "sd")
            nc.scalar.activation(sd[:], ss[:], AF.Sqrt, bias=ecol[:], scale=1.0 / H)
            rstd = spool.tile([P, 1], F32, tag=tg + "rstd")
            nc.vector.reciprocal(rstd[:], sd[:])
            nc.vector.tensor_scalar(xc[:], xc[:], rstd[:], None, ALU.mult)
            nc.vector.tensor_tensor(out=xc[:], in0=xc[:], in1=lng_t[:], op=ALU.mult)
            if out_f32 is not None:
                nc.vector.tensor_tensor(out=out_f32, in0=xc[:], in1=lnb_t[:], op=ALU.add)
                if out_bf is not None:
                    nc.vector.tensor_copy(out_bf, out_f32)
            else:
                nc.vector.tensor_tensor(out=out_bf, in0=xc[:], in1=lnb_t[:], op=ALU.add)

        def pe_transpose_to(dst, src128, ident, tp_pool, dtype):
            t = tp_pool.tile([P, P], dtype, tag="tp")
            nc.tensor.matmul(t[:], src128, ident[:], is_transpose=True)
            nc.vector.tensor_copy(dst, t[:])

        # ================= Phase 1: LN1 + transpose =================
        xnorm_bf = gpool.tile([BC, H], BF16, tag="xnorm_bf")
        layer_norm(x_t[:], 0, None, xnorm_bf[:])

        xnT = gpool.tile([P, KH, BC], BF16, tag="xnT")
        with tc.tile_pool(name="tp1", bufs=2, space="PSUM") as tp1:
            for k in range(KH):
                pe_transpose_to(xnT[:, k, :], xnorm_bf[:, ts(k, P)], ident_bf, tp1, BF16)

            # ============= Phase 2: LSTM gates =============
            si = gpool.tile([BC, H], F32, tag="si")
            sf = gpool.tile([BC, H], F32, tag="sf")
            tg_ = gpool.tile([BC, H], F32, tag="tg")
            so = gpool.tile([BC, H], F32, tag="so")
            gate_sb = [si, sf, tg_, so]
            with tc.tile_pool(name="gpsum", bufs=4, space="PSUM") as gpsum:
                for half in range(2):
                    pss = [gpsum.tile([P, 512], F32, tag="g", name=f"gps{half}_{i}") for i in range(4)]
                    for k in range(KH):
                        for n in range(4):
                            nn = half * 4 + n
                            wt = wstream.tile([P, 512], BF16, tag="w")
                            nc.sync.dma_start(out=wt[:], in_=WihT_d[ts(k, P), ts(nn, 512)])
                            nc.tensor.matmul(pss[n][:], xnT[:, k, :], wt[:],
                                             start=(k == 0), stop=False)
                    for k in range(KH):
                        for n in range(4):
                            nn = half * 4 + n
                            wt = wstream.tile([P, 512], BF16, tag="w")
                            nc.sync.dma_start(out=wt[:], in_=WhhT_d[ts(k, P), ts(nn, 512)])
                            nc.tensor.matmul(pss[n][:], h0T_t[:, k, :], wt[:],
                                             start=False, stop=False)
                    for n in range(4):
                        nn = half * 4 + n
                        nc.tensor.matmul(pss[n][:], ones_f[:],
                                         biasg_t[:, ts(nn, 512)],
                                         start=False, stop=True)
                        gate = nn // 2   # 0:i 1:f 2:g 3:o
                        func = AF.Tanh if gate == 2 else AF.Sigmoid
                        nc.scalar.activation(gate_sb[gate][:, ts(nn % 2, 512)],
                                             pss[n][:], func, bias=zcol[:])

            # LSTM cell elementwise
            cn_t = gpool.tile([BC, H], F32, tag="cn")
            nc.vector.tensor_tensor(out=cn_t[:], in0=sf[:], in1=c0_t[:], op=ALU.mult)
            t2 = gpool.tile([BC, H], F32, tag="scratch2", name="t2")
            nc.vector.tensor_tensor(out=t2[:], in0=si[:], in1=tg_[:], op=ALU.mult)
            nc.vector.tensor_tensor(out=cn_t[:], in0=cn_t[:], in1=t2[:], op=ALU.add)
            nc.sync.dma_start(out=cnew_d[:], in_=cn_t[:])
            tcn = gpool.tile([BC, H], F32, tag="scratch2", name="tcn")
            nc.scalar.activation(tcn[:], cn_t[:], AF.Tanh, bias=zcol[:])
            newh_t = gpool.tile([BC, H], F32, tag="newh")
            nc.vector.tensor_tensor(out=newh_t[:], in0=so[:], in1=tcn[:], op=ALU.mult)
            nc.vector.tensor_tensor(out=newh_t[:], in0=x_t[:], in1=newh_t[:], op=ALU.add)
            nc.sync.dma_start(out=newh_d[:], in_=newh_t[:])

            # ============= Phase 3: LN2 + transpose =============
            hnorm_f = gpool.tile([BC, H], F32, tag="hnorm_f")
            hnorm_bf = gpool.tile([BC, H], BF16, tag="hnorm_bf")
            layer_norm(newh_t[:], 1, hnorm_f[:], hnorm_bf[:])
            hnT = gpool.tile([P, KH, BC], BF16, tag="hnT")
            for k in range(KH):
                pe_transpose_to(hnT[:, k, :], hnorm_bf[:, ts(k, P)], ident_bf, tp1, BF16)

        # ============= Phase 4: qT = Wa @ h_norm^T + Wa_b (transposed layout) ====
        qT = gpool.tile([P, KH, BC], F32, tag="si", name="qT")
        with tc.tile_pool(name="qpsum", bufs=2, space="PSUM") as qpsum:
            for m in range(KH):
                ps = qpsum.tile([P, BC], F32, tag="q")
                for k in range(KH):
                    wt = wstream.tile([P, P], BF16, tag="w", name="wa_t")
                    nc.sync.dma_start(out=wt[:], in_=WaT_d[ts(k, P), ts(m, P)])
                    nc.tensor.matmul(ps[:], wt[:], hnT[:, k, :],
                                     start=(k == 0), stop=False)
                nc.tensor.matmul(ps[:], wab_t[:, ts(m, P)], ones_f[:],
                                 start=False, stop=True)
                nc.vector.tensor_copy(qT[:, m, :], ps[:])

        # ============= Phase 5: attention scores =============
        with tc.tile_pool(name="ktpsum", bufs=5, space="PSUM") as ktpsum, \
             tc.tile_pool(name="scpsum", bufs=2, space="PSUM") as scpsum:
            for g in range(NG):
                kts = []
                for k2 in range(KH // 2):
                    kt = ktpool.tile([P, 2, GB, S], F8, tag="kt", name="kt")
                    nc.sync.dma_start(
                        out=kt[:],
                        in_=keysT_d[g, ts(k2, 2 * P), :, :].rearrange(
                            "(two p) g s -> p two g s", two=2))
                    kts.append(kt)
                sc_ps = scpsum.tile([1, GB * S], F32, tag="sc")
                for m2 in range(KH // 2):
                    e_m2 = epool.tile([P, 2, GB, S], F8, tag="e", name="e_m2")
                    for mh in range(2):
                        m = m2 * 2 + mh
                        ps = ktpsum.tile([P, GB * S], F32, tag="ktp")
                        for k2 in range(KH // 2):
                            nc.tensor.matmul(ps[:], UaT_sb[:, ts(k2, 2), ts(m, P)],
                                             kts[k2][:],
                                             start=(k2 == 0), stop=(k2 == KH // 2 - 1),
                                             perf_mode=mybir.MatmulPerfMode.DoubleRow)
                        for j in range(GB):
                            b = g * GB + j
                            nc.scalar.activation(e_m2[:, mh, j, :], ps[:, ts(j, S)],
                                                 AF.Tanh, bias=qT[:, m, b:b + 1])
                    nc.tensor.matmul(sc_ps[:], va_t[:, ts(m2, 2), 0:1], e_m2[:],
                                     start=(m2 == 0), stop=(m2 == KH // 2 - 1),
                                     perf_mode=mybir.MatmulPerfMode.DoubleRow)
                scb = opool.tile([1, GB * S], F32, tag="scb", name="scb")
                nc.vector.tensor_copy(scb[:], sc_ps[:])
                nc.sync.dma_start(out=sc_scratch_d[ts(g, GB * S)], in_=scb[:])

        # ============= Phase 6: softmax =============
        sc2 = gpool.tile([BC, S], F32, tag="sc2")
        nc.sync.dma_start(out=sc2[:], in_=sc_scratch_d[:].rearrange("(b s) -> b s", b=BC))
        mx = spool.tile([P, 1], F32, tag="mx")
        nc.vector.tensor_reduce(out=mx[:], in_=sc2[:], axis=mybir.AxisListType.X,
                                op=ALU.max)
        nmx = spool.tile([P, 1], F32, tag="nmx")
        nc.vector.tensor_scalar_mul(nmx[:], mx[:], -1.0)
        wsm = gpool.tile([BC, S], F32, tag="wsm")
        sume = spool.tile([P, 1], F32, tag="sume")
        nc.scalar.activation(wsm[:], sc2[:], AF.Exp, bias=nmx[:], accum_out=sume[:])
        rse = spool.tile([P, 1], F32, tag="rse")
        nc.vector.reciprocal(rse[:], sume[:])
        wsm_bf = gpool.tile([BC, S], BF16, tag="wsm_bf")
        nc.vector.tensor_scalar(wsm_bf[:], wsm[:], rse[:], None, ALU.mult)
        wT_sb = gpool.tile([S, BC], BF16, tag="wT")
        with tc.tile_pool(name="tp2", bufs=2, space="PSUM") as tp2:
            pe_transpose_to(wT_sb[:], wsm_bf[:], ident_bf, tp2, BF16)

        # ============= Phase 7: context =============
        ctxT = gpool.tile([P, KH, BC], F32, tag="sf", name="ctxT")
        with tc.tile_pool(name="cpsum", bufs=1, space="PSUM") as cpsum:
            cts = [cpsum.tile([P, BC], F32, tag=f"ct{m}", name=f"ct{m}") for m in range(KH)]
            for bb in range(NG):
                kn4 = knpool.tile([S, GB, H], F8, tag="kn", name="kn4")
                nc.gpsimd.dma_start(out=kn4[:],
                                    in_=keysN_d[ts(bb, GB), :, :].rearrange("g s h -> s g h"))
                for j in range(GB):
                    b = bb * GB + j
                    for m in range(KH):
                        nc.tensor.matmul(cts[m][:, b:b + 1], kn4[:, j, ts(m, P)],
                                         wT_sb[:, b:b + 1], start=True, stop=True)
            for m in range(KH):
                nc.vector.tensor_copy(ctxT[:, m, :], cts[m][:])

        ctxn = gpool.tile([BC, H], F32, tag="tg", name="ctxn")
        with tc.tile_pool(name="tp3", bufs=2, space="PSUM") as tp3:
            for m in range(KH):
                pe_transpose_to(ctxn[:, ts(m, P)], ctxT[:, m, :], ident_f, tp3, F32)

            # ============= Phase 8: LN3 =============
            nc.vector.tensor_tensor(out=ctxn[:], in0=hnorm_f[:], in1=ctxn[:],
                                    op=ALU.add)
            attnh_bf = gpool.tile([BC, H], BF16, tag="attnh_bf")
            layer_norm(ctxn[:], 2, None, attnh_bf[:])
            ahT = gpool.tile([P, KH, BC], BF16, tag="ahT")
            for k in range(KH):
                pe_transpose_to(ahT[:, k, :], attnh_bf[:, ts(k, P)], ident_bf, tp3, BF16)

        # ============= Phase 9: SwiGLU FFN =============
        NF = F // 512  # 8 chunks of the ffn dim
        with tc.tile_pool(name="fpsum", bufs=1, space="PSUM") as fpsum, \
             tc.tile_pool(name="upsum", bufs=2, space="PSUM") as upsum, \
             tc.tile_pool(name="tpsum", bufs=2, space="PSUM") as tpsum:
            ffd = [fpsum.tile([P, 512], F32, tag=f"ffd{h2}", name=f"ffd{h2}") for h2 in range(2)]
            for n in range(NF):
                u1 = upsum.tile([P, 512], F32, tag="u1")
                u3 = upsum.tile([P, 512], F32, tag="u3")
                for k in range(KH):
                    wt = wstream.tile([P, 512], BF16, tag="w")
                    nc.scalar.dma_start(out=wt[:], in_=w1T_d[ts(k, P), ts(n, 512)])
                    nc.tensor.matmul(u1[:], ahT[:, k, :], wt[:],
                                     start=(k == 0), stop=(k == KH - 1))
                for k in range(KH):
                    wt = wstream.tile([P, 512], BF16, tag="w")
                    nc.scalar.dma_start(out=wt[:], in_=w3T_d[ts(k, P), ts(n, 512)])
                    nc.tensor.matmul(u3[:], ahT[:, k, :], wt[:],
                                     start=(k == 0), stop=(k == KH - 1))
                sg = opool.tile([P, 512], F32, tag="sg")
                nc.scalar.activation(sg[:], u1[:], AF.Sigmoid, bias=zcol[:])
                v1 = opool.tile([P, 512], F32, tag="v1")
                nc.vector.tensor_tensor(out=v1[:], in0=u1[:], in1=sg[:], op=ALU.mult)
                v_bf = opool.tile([P, 512], BF16, tag="v_bf")
                nc.vector.tensor_tensor(out=v_bf[:], in0=v1[:], in1=u3[:], op=ALU.mult)
                for c in range(4):
                    f_idx = n * 4 + c
                    tp = tpsum.tile([P, P], BF16, tag="vtp")
                    nc.tensor.matmul(tp[:], v_bf[:, ts(c, P)], ident_bf[:],
                                     is_transpose=True)
                    vT = vtpool.tile([P, P], BF16, tag="vT")
                    nc.vector.tensor_copy(vT[:], tp[:])
                    for h2 in range(2):
                        wt = wstream.tile([P, 512], BF16, tag="w")
                        nc.scalar.dma_start(out=wt[:], in_=w2T_d[ts(f_idx, P), ts(h2, 512)])
                        nc.tensor.matmul(ffd[h2][:], vT[:], wt[:],
                                         start=(f_idx == 0), stop=(f_idx == F // P - 1),
                                         skip_group_check=True)
            for h2 in range(2):
                ot = opool.tile([P, 512], F32, tag="ot")
                nc.vector.tensor_tensor(out=ot[:], in0=ffd[h2][:],
                                        in1=newh_t[:, ts(h2, 512)], op=ALU.add)
                nc.sync.dma_start(out=out_d[:, ts(h2, 512)], in_=ot[:])

    nc.compile()
    return nc


def _va_pad(va):
    v = np.zeros((P, KH, 16), np.float32)
    v[:, :, 0] = va[0].reshape(KH, P).T
    return np.ascontiguousarray(v.astype(mybir.dt.np(F8)))


def prepare_inputs(inputs):
    """Host-side slicing / transposition / dtype casts. Returns per-core in_maps."""
    bf = ml_dtypes.bfloat16
    f32 = np.float32

    def c(a, dtype):
        return np.ascontiguousarray(a, dtype=dtype)

    x = np.asarray(inputs["x"], f32)
    h0 = np.asarray(inputs["h0"], f32)
    c0 = np.asarray(inputs["c0"], f32)
    keys = np.asarray(inputs["keys"], f32)

    shared = {
        "WihT": c(np.asarray(inputs["W_ih"]).T, bf),
        "WhhT": c(np.asarray(inputs["W_hh"]).T, bf),
        "biasg": c((np.asarray(inputs["b_ih"]) + np.asarray(inputs["b_hh"]))[None, :], bf),
        "WaT": c(np.asarray(inputs["Wa"]).T, bf),
        "Wab": c(np.asarray(inputs["Wa_b"])[None, :], bf),
        "UaT": c(np.asarray(inputs["Ua"]).T, mybir.dt.np(F8)),
        "Va8": _va_pad(np.asarray(inputs["Va"])),
        "w1T": c(np.asarray(inputs["w1"]).T, bf),
        "w3T": c(np.asarray(inputs["w3"]).T, bf),
        "w2T": c(np.asarray(inputs["w2"]).T, bf),
    }
    for i in (1, 2, 3):
        shared[f"ln{i}g"] = c(np.broadcast_to(np.asarray(inputs[f"ln{i}_g"]), (P, H)), f32)
        shared[f"ln{i}b"] = c(np.broadcast_to(np.asarray(inputs[f"ln{i}_b"]), (P, H)), f32)

    in_maps = []
    for core in range(NCORES):
        sl = slice(core * BC, (core + 1) * BC)
        kc = keys[sl]                                    # [BC, S, H]
        keysT = kc.reshape(NG, GB, S, H).transpose(0, 3, 1, 2)   # [NG, H, GB, S]
        m = dict(shared)
        m["x"] = c(x[sl], f32)
        m["h0T"] = c(h0[sl].T, bf)
        m["c0"] = c(c0[sl], f32)
        m["keysT"] = c(keysT, mybir.dt.np(F8))
        m["keysN"] = c(kc, mybir.dt.np(F8))
        in_maps.append(m)
    return in_maps


_PROGRAM = None


def kernel(**inputs):
    global _PROGRAM, LAST_RESULT
    if _PROGRAM is None:
        _PROGRAM = build_program()
    in_maps = prepare_inputs(inputs)
    res = run_bass_kernel_spmd(_PROGRAM, in_maps, list(range(NCORES)), trace=TRACE)
    LAST_RESULT = res
    outs = np.concatenate([np.asarray(r["out"]) for r in res.results], axis=0)
    newh = np.concatenate([np.asarray(r["new_h"]) for r in res.results], axis=0)
    cnew = np.concatenate([np.asarray(r["c_new"]) for r in res.results], axis=0)
    return outs.astype(np.float32), (newh.astype(np.float32), cnew.astype(np.float32))
